# revision 6
# baseline (speedup 1.0000x reference)
"""GCN-LSTM layer on 8 Trainium2 NeuronCores.

Strategy (graph/data parallel, no collectives):
  - Nodes sharded 8 x 6250 by dst ownership; `feature` replicated per core as
    the gather tables; LSTM weights replicated.
  - Host prep (index manipulation only): edges sorted by (dst-tile, src-half),
    bucketed per (core, 128-dst tile), padded to SPMD-identical chunk counts;
    per-edge mean weights 1/max(deg,1); own-slice features pre-transposed.
  - Device: batched dma_gather fetches source rows for runs of 128-edge
    chunks (int16 indices over two half-tables, since idx is int16); per
    chunk a scaled one-hot [128 edges, 128 dst] is built via
    iota/is_equal/mult and a TensorE matmul accumulates the mean-aggregation
    in PSUM.  Aggregates are transposed on TensorE into x^T layout
    [feat, nodes]; 3 LSTM layers (h0=c0=0 => f-gate and W_hh terms vanish)
    run as weight-stationary float32r matmuls [gates, nodes] with fused
    bias+sigmoid/tanh on ScalarE.
"""

import numpy as np

N = 50000
E = 800000
D = 256
L = 3
NCORES = 8
NPC = N // NCORES            # nodes per core = 6250
NODE_CHUNK = 512
N_NODE_CHUNKS = (NPC + NODE_CHUNK - 1) // NODE_CHUNK     # 13
NPAD = N_NODE_CHUNKS * NODE_CHUNK                        # 6656
TILES = NPAD // 128                                      # 52 dst tiles / core
SPLIT = 32768                # feature table split so local idx fits int16
M_G = 8                      # max chunks per dma_gather (1024 idxs = SWDGE ring cap)
G_NBUF = 3
MM_DT = "float32r"           # matmul compute dtype

_cache = {}


def _prep_direction(src, dst):
    """Sort edges by (owning 128-dst tile, src-half), pad each tile's lo/hi
    chunk counts to the max over cores (SPMD program identity).

    Returns per-core arrays:
      idx16  [NCORES, 128, NCHUNK*8] int16  (dma_gather layout: flat edge
              slot i lives at [16*g + i%16, i//16] for all g in 0..7)
      dstloc [NCORES, 128, NCHUNK] f32  (dst within tile, -1 for padding)
      wgt    [NCORES, 128, NCHUNK] f32  (1/max(deg,1), 0 for padding)
    plus shared chunk metadata: chunks_per_tile list and per-chunk table
    tags (0=lo table, 1=hi table).
    """
    deg = np.bincount(dst, minlength=N)
    w_node = (1.0 / np.maximum(deg, 1)).astype(np.float32)

    core = dst // NPC
    tloc = (dst - core * NPC) // 128
    tkey = core * TILES + tloc
    hi = (src >= SPLIT).astype(np.int8)
    order = np.lexsort((hi, tkey))
    src_s = src[order]
    dst_s = dst[order]
    tkey_s = tkey[order]
    hi_s = hi[order]
    w_s = w_node[dst_s]

    bounds = np.searchsorted(tkey_s, np.arange(NCORES * TILES + 1))
    hicum = np.concatenate([[0], np.cumsum(hi_s)])
    n_tot = (bounds[1:] - bounds[:-1]).reshape(NCORES, TILES)
    n_hi = (hicum[bounds[1:]] - hicum[bounds[:-1]]).reshape(NCORES, TILES)
    n_lo = n_tot - n_hi

    ch_lo = (-(-n_lo // 128)).max(axis=0)
    ch_hi = (-(-n_hi // 128)).max(axis=0)
    empty = (ch_lo + ch_hi) == 0
    ch_lo[empty] = 1
    nchunk = int((ch_lo + ch_hi).sum())

    tags = []
    for t in range(TILES):
        tags += [0] * int(ch_lo[t]) + [1] * int(ch_hi[t])
    chunks_per_tile = [int(ch_lo[t] + ch_hi[t]) for t in range(TILES)]

    slot_starts = np.concatenate([[0], np.cumsum((ch_lo + ch_hi) * 128)])
    srcloc = np.zeros((NCORES, nchunk * 128), np.int32)
    dstloc = np.full((NCORES, nchunk * 128), -1.0, np.float32)
    wgt = np.zeros((NCORES, nchunk * 128), np.float32)
    for c in range(NCORES):
        base = c * NPC
        for t in range(TILES):
            e0 = bounds[c * TILES + t]
            nl = int(n_lo[c, t])
            nh = int(n_hi[c, t])
            s_lo = int(slot_starts[t])
            s_hi = s_lo + int(ch_lo[t]) * 128
            for (es, n, ss, off) in ((e0, nl, s_lo, 0), (e0 + nl, nh, s_hi, SPLIT)):
                if n == 0:
                    continue
                srcloc[c, ss:ss + n] = src_s[es:es + n] - off
                dstloc[c, ss:ss + n] = (dst_s[es:es + n] - base - t * 128).astype(np.float32)
                wgt[c, ss:ss + n] = w_s[es:es + n]

    # [128, NCHUNK] with [p, ci] = edge slot ci*128+p
    def interleave(a):
        return np.ascontiguousarray(a.reshape(NCORES, nchunk, 128).transpose(0, 2, 1))

    dstloc = interleave(dstloc)
    wgt = interleave(wgt)
    assert srcloc.max() < 32768
    # dma_gather int16 layout: flat slot i at [i%16, i//16], tiled to 128 rows
    base16 = srcloc.reshape(NCORES, nchunk * 8, 16).transpose(0, 2, 1).astype(np.int16)
    idx16 = np.ascontiguousarray(np.tile(base16, (1, 8, 1)))
    return idx16, dstloc, wgt, chunks_per_tile, tags


def _make_groups(tags):
    """Runs of same-table chunks, capped at M_G: list of (start, n, tag)."""
    groups = []
    i = 0
    while i < len(tags):
        j = i
        while j < len(tags) and tags[j] == tags[i] and j - i < M_G:
            j += 1
        groups.append((i, j - i, tags[i]))
        i = j
    return groups


def _keep_rows(w):
    # PyTorch gate order i,f,g,o; f unused when c0=0 -> keep i,g,o
    return np.concatenate([w[0:256], w[512:1024]], axis=0)


def _build_program(cfg):
    import contextlib
    import concourse.tile as tile
    from concourse import bacc, mybir
    from concourse.masks import make_identity

    f32 = mybir.dt.float32
    i16 = mybir.dt.int16
    i32 = mybir.dt.int32
    mmdt = getattr(mybir.dt, MM_DT)

    nchunks, chunks, tags = cfg["nchunks"], cfg["chunks"], cfg["tags"]
    groups = [_make_groups(tags[di]) for di in range(2)]
    # chunk -> (group index, slot in group, is_first_chunk_of_group)
    ch2grp = []
    for di in range(2):
        m = {}
        for gi, (st, n, _tag) in enumerate(groups[di]):
            for k in range(n):
                m[st + k] = (gi, k, k == 0)
        ch2grp.append(m)

    nc = bacc.Bacc("TRN2", target_bir_lowering=False)

    flo_d = nc.dram_tensor("feat_lo", [SPLIT, D], mmdt, kind="ExternalInput")
    fhi_d = nc.dram_tensor("feat_hi", [N - SPLIT, D], mmdt, kind="ExternalInput")
    featT_d = nc.dram_tensor("featT", [D, NPAD], mmdt, kind="ExternalInput")
    idx_d, dst_d, wgt_d = [], [], []
    for di in range(2):
        idx_d.append(nc.dram_tensor(f"idx{di}", [128, nchunks[di] * 8], i16, kind="ExternalInput"))
        dst_d.append(nc.dram_tensor(f"dstloc{di}", [128, nchunks[di]], f32, kind="ExternalInput"))
        wgt_d.append(nc.dram_tensor(f"wgt{di}", [128, nchunks[di]], f32, kind="ExternalInput"))
    wT_d = [nc.dram_tensor("W0T", [768, 768], mmdt, kind="ExternalInput"),
            nc.dram_tensor("W1T", [D, 768], mmdt, kind="ExternalInput"),
            nc.dram_tensor("W2T", [D, 768], mmdt, kind="ExternalInput")]
    bih_d = [nc.dram_tensor(f"bih{l}", [128, 6], f32, kind="ExternalInput") for l in range(L)]
    bhh_d = [nc.dram_tensor(f"bhh{l}", [128, 6], f32, kind="ExternalInput") for l in range(L)]
    hsT_d = nc.dram_tensor("hsT", [L, D, NPAD], mmdt, kind="ExternalOutput")
    csT_d = nc.dram_tensor("csT", [L, D, NPAD], f32, kind="ExternalOutput")

    Sig = mybir.ActivationFunctionType.Sigmoid
    Tanh = mybir.ActivationFunctionType.Tanh

    with tile.TileContext(nc) as tc, contextlib.ExitStack() as ctx:
        const_p = ctx.enter_context(tc.tile_pool(name="const", bufs=1))
        g_p = ctx.enter_context(tc.tile_pool(name="g", bufs=1))
        oh_p = ctx.enter_context(tc.tile_pool(name="oh", bufs=4))
        hin_p = ctx.enter_context(tc.tile_pool(name="hin", bufs=3))
        xt_p = ctx.enter_context(tc.tile_pool(name="xt", bufs=2))
        gate_p = ctx.enter_context(tc.tile_pool(name="gate", bufs=7))
        cth_p = ctx.enter_context(tc.tile_pool(name="cth", bufs=10))
        agg_ps = ctx.enter_context(tc.tile_pool(name="aggps", bufs=2, space="PSUM"))
        tr_ps = ctx.enter_context(tc.tile_pool(name="trps", bufs=2, space="PSUM"))
        mm_ps = ctx.enter_context(tc.tile_pool(name="mmps", bufs=3, space="PSUM"))

        ident = const_p.tile([128, 128], f32, tag="ident")
        make_identity(nc, ident[:])
        iota_i = const_p.tile([128, 128], i32, tag="iota_i")
        nc.gpsimd.iota(iota_i[:], pattern=[[1, 128]], base=0, channel_multiplier=0)
        iota_f = const_p.tile([128, 128], f32, tag="iota_f")
        nc.vector.tensor_copy(iota_f[:], iota_i[:])

        # LSTM weights -> SBUF, k-tile major: w[l][:, k*768 + gt*128 :]
        w_sb = []
        for l in range(L):
            kt = 6 if l == 0 else 2
            w = const_p.tile([128, kt * 768], mmdt, name=f"w{l}", tag=f"w{l}")
            for k in range(kt):
                nc.sync.dma_start(w[:, k * 768:(k + 1) * 768], wT_d[l][k * 128:(k + 1) * 128, :])
            w_sb.append(w)
        b_sb = []
        for l in range(L):
            bi = const_p.tile([128, 6], f32, name=f"bi{l}", tag=f"bi{l}")
            nc.sync.dma_start(bi[:], bih_d[l][:])
            bh = const_p.tile([128, 6], f32, name=f"bh{l}", tag=f"bh{l}")
            nc.sync.dma_start(bh[:], bhh_d[l][:])
            b = const_p.tile([128, 6], f32, name=f"b{l}", tag=f"b{l}")
            nc.vector.tensor_add(b[:], bi[:], bh[:])
            b_sb.append(b)

        idx_sb, dst_sb, wgt_sb = [], [], []
        for di in range(2):
            s = const_p.tile([128, nchunks[di] * 8], i16, name=f"idx{di}", tag=f"idx{di}")
            nc.sync.dma_start(s[:], idx_d[di][:])
            idx_sb.append(s)
            dl = const_p.tile([128, nchunks[di]], f32, name=f"dst{di}", tag=f"dst{di}")
            nc.sync.dma_start(dl[:], dst_d[di][:])
            dst_sb.append(dl)
            w = const_p.tile([128, nchunks[di]], f32, name=f"wgt{di}", tag=f"wgtsb{di}")
            nc.sync.dma_start(w[:], wgt_d[di][:])
            wgt_sb.append(w)

        G_bufs = [g_p.tile([128, M_G, D], mmdt, tag=f"G{i}", name=f"G{i}") for i in range(G_NBUF)]

        def emit_gather(di, gi):
            st, n, tag = groups[di][gi]
            table = flo_d if tag == 0 else fhi_d
            nc.gpsimd.dma_gather(
                out_ap=G_bufs[gi % G_NBUF][:, :n, :],
                in_ap=table[:],
                idxs_ap=idx_sb[di][:, st * 8:(st + n) * 8],
                num_idxs=n * 128,
                num_idxs_reg=n * 128,
                elem_size=D,
            )

        ci_d = [0, 0]
        for nci in range(N_NODE_CHUNKS):
            # ---- Phase A: aggregation for this chunk's 4 dst tiles ----
            xt = [xt_p.tile([128, NODE_CHUNK], mmdt, tag=f"xt{k}", name=f"xt{k}_{nci}")
                  for k in range(6)]
            for di in range(2):
                for tt in range(4):
                    tg = nci * 4 + tt
                    psum = agg_ps.tile([128, D], f32, name=f"aggps_{nci}_{di}_{tt}", tag="aggps")
                    nch = chunks[di][tg]
                    for k in range(nch):
                        ci = ci_d[di]
                        gi, slot, first = ch2grp[di][ci]
                        if first:
                            emit_gather(di, gi)
                        oh = oh_p.tile([128, 128], mmdt, tag="oh", name=f"oh_{di}_{ci}")
                        nc.any.tensor_scalar(
                            out=oh[:], in0=iota_f[:],
                            scalar1=dst_sb[di][:, ci:ci + 1],
                            scalar2=wgt_sb[di][:, ci:ci + 1],
                            op0=mybir.AluOpType.is_equal, op1=mybir.AluOpType.mult,
                        )
                        nc.tensor.matmul(
                            out=psum[:],
                            lhsT=oh[:],
                            rhs=G_bufs[gi % G_NBUF][:, slot, :],
                            start=(k == 0), stop=(k == nch - 1),
                        )
                        ci_d[di] += 1
                    hin = hin_p.tile([128, D], f32, tag="hin", name=f"hin_{nci}_{di}_{tt}")
                    nc.any.tensor_copy(hin[:], psum[:])
                    for h in range(2):
                        pst = tr_ps.tile([128, 128], f32, name=f"trps_{nci}_{di}_{tt}_{h}", tag="trps")
                        nc.tensor.transpose(pst[:], hin[:, h * 128:(h + 1) * 128], ident[:])
                        nc.any.tensor_copy(xt[di * 2 + h][:, tt * 128:(tt + 1) * 128], pst[:])
            for h in range(2):
                nc.sync.dma_start(
                    xt[4 + h][:],
                    featT_d[h * 128:(h + 1) * 128, nci * NODE_CHUNK:(nci + 1) * NODE_CHUNK])

            # ---- Phase B: 3 stacked LSTM cells (h0=c0=0) ----
            x = xt  # 6 k-tiles for layer 0
            for l in range(L):
                kt = 6 if l == 0 else 2
                gates = []
                for gt in range(6):
                    ps = mm_ps.tile([128, NODE_CHUNK], f32, name=f"mmps_{nci}_{l}_{gt}", tag="mmps")
                    for k in range(kt):
                        nc.tensor.matmul(
                            out=ps[:],
                            lhsT=w_sb[l][:, k * 768 + gt * 128:k * 768 + (gt + 1) * 128],
                            rhs=x[k][:],
                            start=(k == 0), stop=(k == kt - 1),
                        )
                    gs = gate_p.tile([128, NODE_CHUNK], f32, tag="gate", name=f"gate_{nci}_{l}_{gt}")
                    nc.scalar.activation(gs[:], ps[:], Tanh if gt in (2, 3) else Sig,
                                         bias=b_sb[l][:, gt:gt + 1])
                    gates.append(gs)
                newx = []
                for h in range(2):
                    cs = cth_p.tile([128, NODE_CHUNK], f32, tag="cth", name=f"c_{nci}_{l}_{h}")
                    nc.vector.tensor_mul(cs[:], gates[h][:], gates[2 + h][:])
                    ts = cth_p.tile([128, NODE_CHUNK], f32, tag="cth", name=f"t_{nci}_{l}_{h}")
                    nc.scalar.activation(ts[:], cs[:], Tanh)
                    hs = cth_p.tile([128, NODE_CHUNK], mmdt, tag="hcast", name=f"h_{nci}_{l}_{h}", bufs=6)
                    nc.vector.tensor_mul(hs[:], gates[4 + h][:], ts[:])
                    nc.sync.dma_start(
                        csT_d[l, h * 128:(h + 1) * 128, nci * NODE_CHUNK:(nci + 1) * NODE_CHUNK], cs[:])
                    nc.sync.dma_start(
                        hsT_d[l, h * 128:(h + 1) * 128, nci * NODE_CHUNK:(nci + 1) * NODE_CHUNK], hs[:])
                    newx.append(hs)
                x = newx

        assert ci_d[0] == nchunks[0] and ci_d[1] == nchunks[1]

    nc.compile()
    return nc


def kernel(feature, edge_src, edge_dst, edge_src_rev, edge_dst_rev, h0, c0,
           W_ih0, W_hh0, b_ih0, b_hh0,
           W_ih1, W_hh1, b_ih1, b_hh1,
           W_ih2, W_hh2, b_ih2, b_hh2):
    from concourse.bass_utils import run_bass_kernel_spmd

    feature = np.ascontiguousarray(np.asarray(feature, np.float32))
    h0 = np.asarray(h0)
    c0 = np.asarray(c0)
    if np.any(h0) or np.any(c0):
        raise NotImplementedError("kernel specialized for h0=c0=0")

    idxF, dstF, wgtF, chF, tagF = _prep_direction(np.asarray(edge_src, np.int64),
                                                  np.asarray(edge_dst, np.int64))
    idxR, dstR, wgtR, chR, tagR = _prep_direction(np.asarray(edge_src_rev, np.int64),
                                                  np.asarray(edge_dst_rev, np.int64))
    cfg = {"nchunks": (len(tagF), len(tagR)), "chunks": (chF, chR),
           "tags": (tagF, tagR)}

    key = (cfg["nchunks"], tuple(chF), tuple(chR), tuple(tagF), tuple(tagR))
    if key not in _cache:
        _cache[key] = _build_program(cfg)
    nc = _cache[key]

    featT = np.zeros((NCORES, D, NPAD), np.float32)
    for c in range(NCORES):
        featT[c, :, :NPC] = feature[c * NPC:(c + 1) * NPC].T

    Ws = [np.ascontiguousarray(_keep_rows(np.asarray(W_ih0, np.float32)).T),
          np.ascontiguousarray(_keep_rows(np.asarray(W_ih1, np.float32)).T),
          np.ascontiguousarray(_keep_rows(np.asarray(W_ih2, np.float32)).T)]
    bihs = [np.ascontiguousarray(_keep_rows(np.asarray(b, np.float32).reshape(-1, 1))
                                 .reshape(6, 128).T) for b in (b_ih0, b_ih1, b_ih2)]
    bhhs = [np.ascontiguousarray(_keep_rows(np.asarray(b, np.float32).reshape(-1, 1))
                                 .reshape(6, 128).T) for b in (b_hh0, b_hh1, b_hh2)]

    feat_lo = np.ascontiguousarray(feature[:SPLIT])
    feat_hi = np.ascontiguousarray(feature[SPLIT:])

    in_maps = []
    for c in range(NCORES):
        m = {"feat_lo": feat_lo, "feat_hi": feat_hi, "featT": featT[c],
             "idx0": idxF[c], "dstloc0": dstF[c], "wgt0": wgtF[c],
             "idx1": idxR[c], "dstloc1": dstR[c], "wgt1": wgtR[c],
             "W0T": Ws[0], "W1T": Ws[1], "W2T": Ws[2]}
        for l in range(L):
            m[f"bih{l}"] = bihs[l]
            m[f"bhh{l}"] = bhhs[l]
        in_maps.append(m)

    res = run_bass_kernel_spmd(nc, in_maps, core_ids=list(range(NCORES)))

    hs = np.empty((L, N, D), np.float32)
    cs = np.empty((L, N, D), np.float32)
    for c in range(NCORES):
        hs[:, c * NPC:(c + 1) * NPC, :] = res.results[c]["hsT"][:, :, :NPC].transpose(0, 2, 1)
        cs[:, c * NPC:(c + 1) * NPC, :] = res.results[c]["csT"][:, :, :NPC].transpose(0, 2, 1)
    output = hs[2:3].copy()
    return output, hs, cs


# revision 7
# speedup vs baseline: 1.5616x; 1.5616x over previous
"""GCN-LSTM layer on 8 Trainium2 NeuronCores.

Strategy (graph/data parallel, no collectives):
  - Nodes sharded 8 x 6250 by dst ownership; `feature` replicated per core as
    the gather tables; LSTM weights replicated.
  - Host prep (index manipulation only): edges sorted by (dst-tile, src-half),
    bucketed per (core, 128-dst tile), padded to SPMD-identical chunk counts;
    per-edge mean weights 1/max(deg,1); own-slice features pre-transposed.
  - Device: batched dma_gather fetches source rows for runs of 128-edge
    chunks (int16 indices over two half-tables, since idx is int16); per
    chunk a scaled one-hot [128 edges, 128 dst] is built via
    iota/is_equal/mult and a TensorE matmul accumulates the mean-aggregation
    in PSUM.  Aggregates are transposed on TensorE into x^T layout
    [feat, nodes]; 3 LSTM layers (h0=c0=0 => f-gate and W_hh terms vanish)
    run as weight-stationary float32r matmuls [gates, nodes] with fused
    bias+sigmoid/tanh on ScalarE.
"""

import numpy as np

N = 50000
E = 800000
D = 256
L = 3
NCORES = 8
NPC = N // NCORES            # nodes per core = 6250
NODE_CHUNK = 512
N_NODE_CHUNKS = (NPC + NODE_CHUNK - 1) // NODE_CHUNK     # 13
NPAD = N_NODE_CHUNKS * NODE_CHUNK                        # 6656
TILES = NPAD // 128                                      # 52 dst tiles / core
SPLIT = 32768                # feature table split so local idx fits int16
M_G = 8                      # max chunks per dma_gather (1024 idxs = SWDGE ring cap)
G_NBUF = 4
MM_DT = "float32r"           # matmul compute dtype (LSTM path)
AGG_DT = "float16"           # gather-table / aggregation matmul dtype

_cache = {}


def _prep_direction(src, dst):
    """Sort edges by (owning 128-dst tile, src-half), pad each tile's lo/hi
    chunk counts to the max over cores (SPMD program identity).

    Returns per-core arrays:
      idx16  [NCORES, 128, NCHUNK*8] int16  (dma_gather layout: flat edge
              slot i lives at [16*g + i%16, i//16] for all g in 0..7)
      dstloc [NCORES, 128, NCHUNK] f32  (dst within tile, -1 for padding)
      wgt    [NCORES, 128, NCHUNK] f32  (1/max(deg,1), 0 for padding)
    plus shared chunk metadata: chunks_per_tile list and per-chunk table
    tags (0=lo table, 1=hi table).
    """
    deg = np.bincount(dst, minlength=N)
    w_node = (1.0 / np.maximum(deg, 1)).astype(np.float32)

    core = dst // NPC
    tloc = (dst - core * NPC) // 128
    tkey = core * TILES + tloc
    hi = (src >= SPLIT).astype(np.int8)
    order = np.lexsort((hi, tkey))
    src_s = src[order]
    dst_s = dst[order]
    tkey_s = tkey[order]
    hi_s = hi[order]
    w_s = w_node[dst_s]

    bounds = np.searchsorted(tkey_s, np.arange(NCORES * TILES + 1))
    hicum = np.concatenate([[0], np.cumsum(hi_s)])
    n_tot = (bounds[1:] - bounds[:-1]).reshape(NCORES, TILES)
    n_hi = (hicum[bounds[1:]] - hicum[bounds[:-1]]).reshape(NCORES, TILES)
    n_lo = n_tot - n_hi

    ch_lo = (-(-n_lo // 128)).max(axis=0)
    ch_hi = (-(-n_hi // 128)).max(axis=0)
    empty = (ch_lo + ch_hi) == 0
    ch_lo[empty] = 1
    nchunk = int((ch_lo + ch_hi).sum())

    tags = []
    for t in range(TILES):
        tags += [0] * int(ch_lo[t]) + [1] * int(ch_hi[t])
    chunks_per_tile = [int(ch_lo[t] + ch_hi[t]) for t in range(TILES)]

    slot_starts = np.concatenate([[0], np.cumsum((ch_lo + ch_hi) * 128)])
    srcloc = np.zeros((NCORES, nchunk * 128), np.int32)
    dstloc = np.full((NCORES, nchunk * 128), -1.0, np.float32)
    wgt = np.zeros((NCORES, nchunk * 128), np.float32)
    for c in range(NCORES):
        base = c * NPC
        for t in range(TILES):
            e0 = bounds[c * TILES + t]
            nl = int(n_lo[c, t])
            nh = int(n_hi[c, t])
            s_lo = int(slot_starts[t])
            s_hi = s_lo + int(ch_lo[t]) * 128
            for (es, n, ss, off) in ((e0, nl, s_lo, 0), (e0 + nl, nh, s_hi, SPLIT)):
                if n == 0:
                    continue
                srcloc[c, ss:ss + n] = src_s[es:es + n] - off
                dstloc[c, ss:ss + n] = (dst_s[es:es + n] - base - t * 128).astype(np.float32)
                wgt[c, ss:ss + n] = w_s[es:es + n]

    # [128, NCHUNK] with [p, ci] = edge slot ci*128+p
    def interleave(a):
        return np.ascontiguousarray(a.reshape(NCORES, nchunk, 128).transpose(0, 2, 1))

    dstloc = interleave(dstloc)
    wgt = interleave(wgt)
    assert srcloc.max() < 32768
    # dma_gather int16 layout: flat slot i at [i%16, i//16], tiled to 128 rows
    base16 = srcloc.reshape(NCORES, nchunk * 8, 16).transpose(0, 2, 1).astype(np.int16)
    idx16 = np.ascontiguousarray(np.tile(base16, (1, 8, 1)))
    return idx16, dstloc, wgt, chunks_per_tile, tags


def _make_groups(tags):
    """Runs of same-table chunks, capped at M_G: list of (start, n, tag)."""
    groups = []
    i = 0
    while i < len(tags):
        j = i
        while j < len(tags) and tags[j] == tags[i] and j - i < M_G:
            j += 1
        groups.append((i, j - i, tags[i]))
        i = j
    return groups


def _keep_rows(w):
    # PyTorch gate order i,f,g,o; f unused when c0=0 -> keep i,g,o
    return np.concatenate([w[0:256], w[512:1024]], axis=0)


def _build_program(cfg):
    import contextlib
    import concourse.tile as tile
    from concourse import bacc, mybir
    from concourse.masks import make_identity

    f32 = mybir.dt.float32
    i16 = mybir.dt.int16
    i32 = mybir.dt.int32
    mmdt = getattr(mybir.dt, MM_DT)
    aggdt = getattr(mybir.dt, AGG_DT)

    nchunks, chunks, tags = cfg["nchunks"], cfg["chunks"], cfg["tags"]
    groups = [_make_groups(tags[di]) for di in range(2)]
    # chunk -> (group index, slot in group, is_first_chunk_of_group)
    ch2grp = []
    for di in range(2):
        m = {}
        for gi, (st, n, _tag) in enumerate(groups[di]):
            for k in range(n):
                m[st + k] = (gi, k, k == 0)
        ch2grp.append(m)

    nc = bacc.Bacc("TRN2", target_bir_lowering=False)

    flo_d = nc.dram_tensor("feat_lo", [SPLIT, D], aggdt, kind="ExternalInput")
    fhi_d = nc.dram_tensor("feat_hi", [N - SPLIT, D], aggdt, kind="ExternalInput")
    featT_d = nc.dram_tensor("featT", [D, NPAD], mmdt, kind="ExternalInput")
    idx_d, dst_d, wgt_d = [], [], []
    for di in range(2):
        idx_d.append(nc.dram_tensor(f"idx{di}", [128, nchunks[di] * 8], i16, kind="ExternalInput"))
        dst_d.append(nc.dram_tensor(f"dstloc{di}", [128, nchunks[di]], f32, kind="ExternalInput"))
        wgt_d.append(nc.dram_tensor(f"wgt{di}", [128, nchunks[di]], f32, kind="ExternalInput"))
    wT_d = [nc.dram_tensor("W0T", [768, 768], mmdt, kind="ExternalInput"),
            nc.dram_tensor("W1T", [D, 768], mmdt, kind="ExternalInput"),
            nc.dram_tensor("W2T", [D, 768], mmdt, kind="ExternalInput")]
    bih_d = [nc.dram_tensor(f"bih{l}", [128, 6], f32, kind="ExternalInput") for l in range(L)]
    bhh_d = [nc.dram_tensor(f"bhh{l}", [128, 6], f32, kind="ExternalInput") for l in range(L)]
    hsT_d = nc.dram_tensor("hsT", [L, D, NPAD], mmdt, kind="ExternalOutput")
    csT_d = nc.dram_tensor("csT", [L, D, NPAD], f32, kind="ExternalOutput")

    Sig = mybir.ActivationFunctionType.Sigmoid
    Tanh = mybir.ActivationFunctionType.Tanh

    with tile.TileContext(nc) as tc, contextlib.ExitStack() as ctx:
        const_p = ctx.enter_context(tc.tile_pool(name="const", bufs=1))
        g_p = ctx.enter_context(tc.tile_pool(name="g", bufs=1))
        oh_p = ctx.enter_context(tc.tile_pool(name="oh", bufs=4))
        hin_p = ctx.enter_context(tc.tile_pool(name="hin", bufs=3))
        xt_p = ctx.enter_context(tc.tile_pool(name="xt", bufs=2))
        gate_p = ctx.enter_context(tc.tile_pool(name="gate", bufs=7))
        cth_p = ctx.enter_context(tc.tile_pool(name="cth", bufs=10))
        agg_ps = ctx.enter_context(tc.tile_pool(name="aggps", bufs=2, space="PSUM"))
        tr_ps = ctx.enter_context(tc.tile_pool(name="trps", bufs=2, space="PSUM"))
        mm_ps = ctx.enter_context(tc.tile_pool(name="mmps", bufs=3, space="PSUM"))

        ident = const_p.tile([128, 128], f32, tag="ident")
        make_identity(nc, ident[:])
        iota_i = const_p.tile([128, 128], i32, tag="iota_i")
        nc.gpsimd.iota(iota_i[:], pattern=[[1, 128]], base=0, channel_multiplier=0)
        iota_f = const_p.tile([128, 128], f32, tag="iota_f")
        nc.vector.tensor_copy(iota_f[:], iota_i[:])

        # LSTM weights -> SBUF, k-tile major: w[l][:, k*768 + gt*128 :]
        w_sb = []
        for l in range(L):
            kt = 6 if l == 0 else 2
            w = const_p.tile([128, kt * 768], mmdt, name=f"w{l}", tag=f"w{l}")
            for k in range(kt):
                nc.sync.dma_start(w[:, k * 768:(k + 1) * 768], wT_d[l][k * 128:(k + 1) * 128, :])
            w_sb.append(w)
        b_sb = []
        for l in range(L):
            bi = const_p.tile([128, 6], f32, name=f"bi{l}", tag=f"bi{l}")
            nc.sync.dma_start(bi[:], bih_d[l][:])
            bh = const_p.tile([128, 6], f32, name=f"bh{l}", tag=f"bh{l}")
            nc.sync.dma_start(bh[:], bhh_d[l][:])
            b = const_p.tile([128, 6], f32, name=f"b{l}", tag=f"b{l}")
            nc.vector.tensor_add(b[:], bi[:], bh[:])
            b_sb.append(b)

        idx_sb, dst_sb, wgt_sb = [], [], []
        for di in range(2):
            s = const_p.tile([128, nchunks[di] * 8], i16, name=f"idx{di}", tag=f"idx{di}")
            nc.sync.dma_start(s[:], idx_d[di][:])
            idx_sb.append(s)
            dl = const_p.tile([128, nchunks[di]], f32, name=f"dst{di}", tag=f"dst{di}")
            nc.sync.dma_start(dl[:], dst_d[di][:])
            dst_sb.append(dl)
            w = const_p.tile([128, nchunks[di]], f32, name=f"wgt{di}", tag=f"wgtsb{di}")
            nc.sync.dma_start(w[:], wgt_d[di][:])
            wgt_sb.append(w)

        G_bufs = [g_p.tile([128, M_G, D], aggdt, tag=f"G{i}", name=f"G{i}") for i in range(G_NBUF)]

        def emit_gather(di, gi):
            st, n, tag = groups[di][gi]
            table = flo_d if tag == 0 else fhi_d
            nc.gpsimd.dma_gather(
                out_ap=G_bufs[gi % G_NBUF][:, :n, :],
                in_ap=table[:],
                idxs_ap=idx_sb[di][:, st * 8:(st + n) * 8],
                num_idxs=n * 128,
                num_idxs_reg=n * 128,
                elem_size=D,
            )

        ci_d = [0, 0]
        for nci in range(N_NODE_CHUNKS):
            # ---- Phase A: aggregation for this chunk's 4 dst tiles ----
            xt = [xt_p.tile([128, NODE_CHUNK], mmdt, tag=f"xt{k}", name=f"xt{k}_{nci}")
                  for k in range(6)]
            for di in range(2):
                for tt in range(4):
                    tg = nci * 4 + tt
                    psum = agg_ps.tile([128, D], f32, name=f"aggps_{nci}_{di}_{tt}", tag="aggps")
                    nch = chunks[di][tg]
                    for k in range(nch):
                        ci = ci_d[di]
                        gi, slot, first = ch2grp[di][ci]
                        if first:
                            emit_gather(di, gi)
                        oh = oh_p.tile([128, 128], aggdt, tag="oh", name=f"oh_{di}_{ci}")
                        nc.any.tensor_scalar(
                            out=oh[:], in0=iota_f[:],
                            scalar1=dst_sb[di][:, ci:ci + 1],
                            scalar2=wgt_sb[di][:, ci:ci + 1],
                            op0=mybir.AluOpType.is_equal, op1=mybir.AluOpType.mult,
                        )
                        nc.tensor.matmul(
                            out=psum[:],
                            lhsT=oh[:],
                            rhs=G_bufs[gi % G_NBUF][:, slot, :],
                            start=(k == 0), stop=(k == nch - 1),
                        )
                        ci_d[di] += 1
                    hin = hin_p.tile([128, D], f32, tag="hin", name=f"hin_{nci}_{di}_{tt}")
                    nc.any.tensor_copy(hin[:], psum[:])
                    for h in range(2):
                        pst = tr_ps.tile([128, 128], f32, name=f"trps_{nci}_{di}_{tt}_{h}", tag="trps")
                        nc.tensor.transpose(pst[:], hin[:, h * 128:(h + 1) * 128], ident[:])
                        nc.any.tensor_copy(xt[di * 2 + h][:, tt * 128:(tt + 1) * 128], pst[:])
            for h in range(2):
                nc.sync.dma_start(
                    xt[4 + h][:],
                    featT_d[h * 128:(h + 1) * 128, nci * NODE_CHUNK:(nci + 1) * NODE_CHUNK])

            # ---- Phase B: 3 stacked LSTM cells (h0=c0=0) ----
            x = xt  # 6 k-tiles for layer 0
            for l in range(L):
                kt = 6 if l == 0 else 2
                gates = []
                for gt in range(6):
                    ps = mm_ps.tile([128, NODE_CHUNK], f32, name=f"mmps_{nci}_{l}_{gt}", tag="mmps")
                    for k in range(kt):
                        nc.tensor.matmul(
                            out=ps[:],
                            lhsT=w_sb[l][:, k * 768 + gt * 128:k * 768 + (gt + 1) * 128],
                            rhs=x[k][:],
                            start=(k == 0), stop=(k == kt - 1),
                        )
                    gs = gate_p.tile([128, NODE_CHUNK], f32, tag="gate", name=f"gate_{nci}_{l}_{gt}")
                    nc.scalar.activation(gs[:], ps[:], Tanh if gt in (2, 3) else Sig,
                                         bias=b_sb[l][:, gt:gt + 1])
                    gates.append(gs)
                newx = []
                for h in range(2):
                    cs = cth_p.tile([128, NODE_CHUNK], f32, tag="cth", name=f"c_{nci}_{l}_{h}")
                    nc.vector.tensor_mul(cs[:], gates[h][:], gates[2 + h][:])
                    ts = cth_p.tile([128, NODE_CHUNK], f32, tag="cth", name=f"t_{nci}_{l}_{h}")
                    nc.scalar.activation(ts[:], cs[:], Tanh)
                    hs = cth_p.tile([128, NODE_CHUNK], mmdt, tag="hcast", name=f"h_{nci}_{l}_{h}", bufs=6)
                    nc.vector.tensor_mul(hs[:], gates[4 + h][:], ts[:])
                    nc.sync.dma_start(
                        csT_d[l, h * 128:(h + 1) * 128, nci * NODE_CHUNK:(nci + 1) * NODE_CHUNK], cs[:])
                    nc.sync.dma_start(
                        hsT_d[l, h * 128:(h + 1) * 128, nci * NODE_CHUNK:(nci + 1) * NODE_CHUNK], hs[:])
                    newx.append(hs)
                x = newx

        assert ci_d[0] == nchunks[0] and ci_d[1] == nchunks[1]

    nc.compile()
    return nc


def kernel(feature, edge_src, edge_dst, edge_src_rev, edge_dst_rev, h0, c0,
           W_ih0, W_hh0, b_ih0, b_hh0,
           W_ih1, W_hh1, b_ih1, b_hh1,
           W_ih2, W_hh2, b_ih2, b_hh2):
    from concourse.bass_utils import run_bass_kernel_spmd

    feature = np.ascontiguousarray(np.asarray(feature, np.float32))
    h0 = np.asarray(h0)
    c0 = np.asarray(c0)
    if np.any(h0) or np.any(c0):
        raise NotImplementedError("kernel specialized for h0=c0=0")

    idxF, dstF, wgtF, chF, tagF = _prep_direction(np.asarray(edge_src, np.int64),
                                                  np.asarray(edge_dst, np.int64))
    idxR, dstR, wgtR, chR, tagR = _prep_direction(np.asarray(edge_src_rev, np.int64),
                                                  np.asarray(edge_dst_rev, np.int64))
    cfg = {"nchunks": (len(tagF), len(tagR)), "chunks": (chF, chR),
           "tags": (tagF, tagR)}

    key = (cfg["nchunks"], tuple(chF), tuple(chR), tuple(tagF), tuple(tagR))
    if key not in _cache:
        _cache[key] = _build_program(cfg)
    nc = _cache[key]

    featT = np.zeros((NCORES, D, NPAD), np.float32)
    for c in range(NCORES):
        featT[c, :, :NPC] = feature[c * NPC:(c + 1) * NPC].T

    Ws = [np.ascontiguousarray(_keep_rows(np.asarray(W_ih0, np.float32)).T),
          np.ascontiguousarray(_keep_rows(np.asarray(W_ih1, np.float32)).T),
          np.ascontiguousarray(_keep_rows(np.asarray(W_ih2, np.float32)).T)]
    bihs = [np.ascontiguousarray(_keep_rows(np.asarray(b, np.float32).reshape(-1, 1))
                                 .reshape(6, 128).T) for b in (b_ih0, b_ih1, b_ih2)]
    bhhs = [np.ascontiguousarray(_keep_rows(np.asarray(b, np.float32).reshape(-1, 1))
                                 .reshape(6, 128).T) for b in (b_hh0, b_hh1, b_hh2)]

    fdt = np.float16 if AGG_DT == "float16" else np.float32
    feat_lo = np.ascontiguousarray(feature[:SPLIT].astype(fdt))
    feat_hi = np.ascontiguousarray(feature[SPLIT:].astype(fdt))

    in_maps = []
    for c in range(NCORES):
        m = {"feat_lo": feat_lo, "feat_hi": feat_hi, "featT": featT[c],
             "idx0": idxF[c], "dstloc0": dstF[c], "wgt0": wgtF[c],
             "idx1": idxR[c], "dstloc1": dstR[c], "wgt1": wgtR[c],
             "W0T": Ws[0], "W1T": Ws[1], "W2T": Ws[2]}
        for l in range(L):
            m[f"bih{l}"] = bihs[l]
            m[f"bhh{l}"] = bhhs[l]
        in_maps.append(m)

    res = run_bass_kernel_spmd(nc, in_maps, core_ids=list(range(NCORES)))

    hs = np.empty((L, N, D), np.float32)
    cs = np.empty((L, N, D), np.float32)
    for c in range(NCORES):
        hs[:, c * NPC:(c + 1) * NPC, :] = res.results[c]["hsT"][:, :, :NPC].transpose(0, 2, 1)
        cs[:, c * NPC:(c + 1) * NPC, :] = res.results[c]["csT"][:, :, :NPC].transpose(0, 2, 1)
    output = hs[2:3].copy()
    return output, hs, cs


# revision 8
# speedup vs baseline: 1.5867x; 1.0160x over previous
"""GCN-LSTM layer on 8 Trainium2 NeuronCores.

Strategy (graph/data parallel, no collectives):
  - Nodes sharded 8 x 6250 by dst ownership; `feature` replicated per core as
    the gather tables; LSTM weights replicated.
  - Host prep (index manipulation only): edges sorted by (dst-tile, src-half),
    bucketed per (core, 128-dst tile), padded to SPMD-identical chunk counts;
    per-edge mean weights 1/max(deg,1); own-slice features pre-transposed.
  - Device: batched dma_gather fetches source rows for runs of 128-edge
    chunks (int16 indices over two half-tables, since idx is int16); per
    chunk a scaled one-hot [128 edges, 128 dst] is built via
    iota/is_equal/mult and a TensorE matmul accumulates the mean-aggregation
    in PSUM.  Aggregates are transposed on TensorE into x^T layout
    [feat, nodes]; 3 LSTM layers (h0=c0=0 => f-gate and W_hh terms vanish)
    run as weight-stationary float32r matmuls [gates, nodes] with fused
    bias+sigmoid/tanh on ScalarE.
"""

import numpy as np

N = 50000
E = 800000
D = 256
L = 3
NCORES = 8
NPC = N // NCORES            # nodes per core = 6250
NODE_CHUNK = 512
N_NODE_CHUNKS = (NPC + NODE_CHUNK - 1) // NODE_CHUNK     # 13
NPAD = N_NODE_CHUNKS * NODE_CHUNK                        # 6656
TILES = NPAD // 128                                      # 52 dst tiles / core
SPLIT = 32768                # feature table split so local idx fits int16
M_G = 8                      # max chunks per dma_gather (1024 idxs = SWDGE ring cap)
G_NBUF = 4
MM_DT = "float32r"           # matmul compute dtype (LSTM path)
AGG_DT = "float16"           # gather-table / aggregation matmul dtype

_cache = {}


def _prep_direction(src, dst):
    """Sort edges by (owning 128-dst tile, src-half), pad each tile's lo/hi
    chunk counts to the max over cores (SPMD program identity).

    Returns per-core arrays:
      idx16  [NCORES, 128, NCHUNK*8] int16  (dma_gather layout: flat edge
              slot i lives at [16*g + i%16, i//16] for all g in 0..7)
      dstloc [NCORES, 128, NCHUNK] f32  (dst within tile, -1 for padding)
      wgt    [NCORES, 128, NCHUNK] f32  (1/max(deg,1), 0 for padding)
    plus shared chunk metadata: chunks_per_tile list and per-chunk table
    tags (0=lo table, 1=hi table).
    """
    deg = np.bincount(dst, minlength=N)
    w_node = (1.0 / np.maximum(deg, 1)).astype(np.float32)

    core = dst // NPC
    tloc = (dst - core * NPC) // 128
    tkey = core * TILES + tloc
    hi = (src >= SPLIT).astype(np.int8)
    order = np.lexsort((hi, tkey))
    src_s = src[order]
    dst_s = dst[order]
    tkey_s = tkey[order]
    hi_s = hi[order]
    w_s = w_node[dst_s]

    bounds = np.searchsorted(tkey_s, np.arange(NCORES * TILES + 1))
    hicum = np.concatenate([[0], np.cumsum(hi_s)])
    n_tot = (bounds[1:] - bounds[:-1]).reshape(NCORES, TILES)
    n_hi = (hicum[bounds[1:]] - hicum[bounds[:-1]]).reshape(NCORES, TILES)
    n_lo = n_tot - n_hi

    ch_lo = (-(-n_lo // 128)).max(axis=0)
    ch_hi = (-(-n_hi // 128)).max(axis=0)
    empty = (ch_lo + ch_hi) == 0
    ch_lo[empty] = 1
    nchunk = int((ch_lo + ch_hi).sum())

    tags = []
    for t in range(TILES):
        tags += [0] * int(ch_lo[t]) + [1] * int(ch_hi[t])
    chunks_per_tile = [int(ch_lo[t] + ch_hi[t]) for t in range(TILES)]

    slot_starts = np.concatenate([[0], np.cumsum((ch_lo + ch_hi) * 128)])
    srcloc = np.zeros((NCORES, nchunk * 128), np.int32)
    dstloc = np.full((NCORES, nchunk * 128), -1.0, np.float32)
    wgt = np.zeros((NCORES, nchunk * 128), np.float32)
    for c in range(NCORES):
        base = c * NPC
        for t in range(TILES):
            e0 = bounds[c * TILES + t]
            nl = int(n_lo[c, t])
            nh = int(n_hi[c, t])
            s_lo = int(slot_starts[t])
            s_hi = s_lo + int(ch_lo[t]) * 128
            for (es, n, ss, off) in ((e0, nl, s_lo, 0), (e0 + nl, nh, s_hi, SPLIT)):
                if n == 0:
                    continue
                srcloc[c, ss:ss + n] = src_s[es:es + n] - off
                dstloc[c, ss:ss + n] = (dst_s[es:es + n] - base - t * 128).astype(np.float32)
                wgt[c, ss:ss + n] = w_s[es:es + n]

    # [128, NCHUNK] with [p, ci] = edge slot ci*128+p
    def interleave(a):
        return np.ascontiguousarray(a.reshape(NCORES, nchunk, 128).transpose(0, 2, 1))

    dstloc = interleave(dstloc)
    wgt = interleave(wgt)
    assert srcloc.max() < 32768
    # dma_gather int16 layout: flat slot i at [i%16, i//16], tiled to 128 rows
    base16 = srcloc.reshape(NCORES, nchunk * 8, 16).transpose(0, 2, 1).astype(np.int16)
    idx16 = np.ascontiguousarray(np.tile(base16, (1, 8, 1)))
    return idx16, dstloc, wgt, chunks_per_tile, tags


def _make_groups(tags):
    """Runs of same-table chunks, capped at M_G: list of (start, n, tag)."""
    groups = []
    i = 0
    while i < len(tags):
        j = i
        while j < len(tags) and tags[j] == tags[i] and j - i < M_G:
            j += 1
        groups.append((i, j - i, tags[i]))
        i = j
    return groups


def _keep_rows(w):
    # PyTorch gate order i,f,g,o; f unused when c0=0 -> keep i,g,o
    return np.concatenate([w[0:256], w[512:1024]], axis=0)


def _build_program(cfg):
    import contextlib
    import concourse.tile as tile
    from concourse import bacc, mybir
    from concourse.masks import make_identity

    f32 = mybir.dt.float32
    i16 = mybir.dt.int16
    i32 = mybir.dt.int32
    mmdt = getattr(mybir.dt, MM_DT)
    aggdt = getattr(mybir.dt, AGG_DT)

    nchunks, chunks, tags = cfg["nchunks"], cfg["chunks"], cfg["tags"]
    groups = [_make_groups(tags[di]) for di in range(2)]
    # chunk -> (group index, slot in group, is_first_chunk_of_group)
    ch2grp = []
    for di in range(2):
        m = {}
        for gi, (st, n, _tag) in enumerate(groups[di]):
            for k in range(n):
                m[st + k] = (gi, k, k == 0)
        ch2grp.append(m)

    nc = bacc.Bacc("TRN2", target_bir_lowering=False)

    flo_d = nc.dram_tensor("feat_lo", [SPLIT, D], aggdt, kind="ExternalInput")
    fhi_d = nc.dram_tensor("feat_hi", [N - SPLIT, D], aggdt, kind="ExternalInput")
    featT_d = nc.dram_tensor("featT", [D, NPAD], mmdt, kind="ExternalInput")
    idx_d, dst_d, wgt_d = [], [], []
    for di in range(2):
        idx_d.append(nc.dram_tensor(f"idx{di}", [128, nchunks[di] * 8], i16, kind="ExternalInput"))
        dst_d.append(nc.dram_tensor(f"dstloc{di}", [128, nchunks[di]], f32, kind="ExternalInput"))
        wgt_d.append(nc.dram_tensor(f"wgt{di}", [128, nchunks[di]], f32, kind="ExternalInput"))
    wT_d = [nc.dram_tensor("W0T", [768, 768], mmdt, kind="ExternalInput"),
            nc.dram_tensor("W1T", [D, 768], mmdt, kind="ExternalInput"),
            nc.dram_tensor("W2T", [D, 768], mmdt, kind="ExternalInput")]
    bih_d = [nc.dram_tensor(f"bih{l}", [128, 6], f32, kind="ExternalInput") for l in range(L)]
    bhh_d = [nc.dram_tensor(f"bhh{l}", [128, 6], f32, kind="ExternalInput") for l in range(L)]
    hsT_d = nc.dram_tensor("hsT", [L, D, NPAD], mmdt, kind="ExternalOutput")
    csT_d = nc.dram_tensor("csT", [L, D, NPAD], f32, kind="ExternalOutput")

    Sig = mybir.ActivationFunctionType.Sigmoid
    Tanh = mybir.ActivationFunctionType.Tanh

    with tile.TileContext(nc) as tc, contextlib.ExitStack() as ctx:
        const_p = ctx.enter_context(tc.tile_pool(name="const", bufs=1))
        g_p = ctx.enter_context(tc.tile_pool(name="g", bufs=1))
        oh_p = ctx.enter_context(tc.tile_pool(name="oh", bufs=6))
        hin_p = ctx.enter_context(tc.tile_pool(name="hin", bufs=3))
        xt_p = ctx.enter_context(tc.tile_pool(name="xt", bufs=2))
        gate_p = ctx.enter_context(tc.tile_pool(name="gate", bufs=7))
        cth_p = ctx.enter_context(tc.tile_pool(name="cth", bufs=10))
        agg_ps = ctx.enter_context(tc.tile_pool(name="aggps", bufs=3, space="PSUM"))
        tr_ps = ctx.enter_context(tc.tile_pool(name="trps", bufs=2, space="PSUM"))
        mm_ps = ctx.enter_context(tc.tile_pool(name="mmps", bufs=3, space="PSUM"))

        ident = const_p.tile([128, 128], f32, tag="ident")
        make_identity(nc, ident[:])
        iota_i = const_p.tile([128, 128], i32, tag="iota_i")
        nc.gpsimd.iota(iota_i[:], pattern=[[1, 128]], base=0, channel_multiplier=0)
        iota_f = const_p.tile([128, 128], aggdt, tag="iota_f")
        nc.vector.tensor_copy(iota_f[:], iota_i[:])

        # LSTM weights -> SBUF, k-tile major: w[l][:, k*768 + gt*128 :]
        w_sb = []
        for l in range(L):
            kt = 6 if l == 0 else 2
            w = const_p.tile([128, kt * 768], mmdt, name=f"w{l}", tag=f"w{l}")
            for k in range(kt):
                nc.sync.dma_start(w[:, k * 768:(k + 1) * 768], wT_d[l][k * 128:(k + 1) * 128, :])
            w_sb.append(w)
        b_sb = []
        for l in range(L):
            bi = const_p.tile([128, 6], f32, name=f"bi{l}", tag=f"bi{l}")
            nc.sync.dma_start(bi[:], bih_d[l][:])
            bh = const_p.tile([128, 6], f32, name=f"bh{l}", tag=f"bh{l}")
            nc.sync.dma_start(bh[:], bhh_d[l][:])
            b = const_p.tile([128, 6], f32, name=f"b{l}", tag=f"b{l}")
            nc.vector.tensor_add(b[:], bi[:], bh[:])
            b_sb.append(b)

        idx_sb, dst_sb, wgt_sb = [], [], []
        for di in range(2):
            s = const_p.tile([128, nchunks[di] * 8], i16, name=f"idx{di}", tag=f"idx{di}")
            nc.sync.dma_start(s[:], idx_d[di][:])
            idx_sb.append(s)
            dl = const_p.tile([128, nchunks[di]], f32, name=f"dst{di}", tag=f"dst{di}")
            nc.sync.dma_start(dl[:], dst_d[di][:])
            dst_sb.append(dl)
            w = const_p.tile([128, nchunks[di]], f32, name=f"wgt{di}", tag=f"wgtsb{di}")
            nc.sync.dma_start(w[:], wgt_d[di][:])
            wgt_sb.append(w)

        G_bufs = [g_p.tile([128, M_G, D], aggdt, tag=f"G{i}", name=f"G{i}") for i in range(G_NBUF)]

        def emit_gather(di, gi):
            st, n, tag = groups[di][gi]
            table = flo_d if tag == 0 else fhi_d
            nc.gpsimd.dma_gather(
                out_ap=G_bufs[gi % G_NBUF][:, :n, :],
                in_ap=table[:],
                idxs_ap=idx_sb[di][:, st * 8:(st + n) * 8],
                num_idxs=n * 128,
                num_idxs_reg=n * 128,
                elem_size=D,
            )

        ci_d = [0, 0]
        for nci in range(N_NODE_CHUNKS):
            # ---- Phase A: aggregation for this chunk's 4 dst tiles ----
            xt = [xt_p.tile([128, NODE_CHUNK], mmdt, tag=f"xt{k}", name=f"xt{k}_{nci}")
                  for k in range(6)]
            for di in range(2):
                for tt in range(4):
                    tg = nci * 4 + tt
                    psum = agg_ps.tile([128, D], f32, name=f"aggps_{nci}_{di}_{tt}", tag="aggps")
                    nch = chunks[di][tg]
                    for k in range(nch):
                        ci = ci_d[di]
                        gi, slot, first = ch2grp[di][ci]
                        if first:
                            emit_gather(di, gi)
                        oh = oh_p.tile([128, 128], aggdt, tag="oh", name=f"oh_{di}_{ci}")
                        nc.any.tensor_scalar(
                            out=oh[:], in0=iota_f[:],
                            scalar1=dst_sb[di][:, ci:ci + 1],
                            scalar2=wgt_sb[di][:, ci:ci + 1],
                            op0=mybir.AluOpType.is_equal, op1=mybir.AluOpType.mult,
                        )
                        nc.tensor.matmul(
                            out=psum[:],
                            lhsT=oh[:],
                            rhs=G_bufs[gi % G_NBUF][:, slot, :],
                            start=(k == 0), stop=(k == nch - 1),
                        )
                        ci_d[di] += 1
                    hin = hin_p.tile([128, D], f32, tag="hin", name=f"hin_{nci}_{di}_{tt}")
                    nc.any.tensor_copy(hin[:], psum[:])
                    for h in range(2):
                        pst = tr_ps.tile([128, 128], f32, name=f"trps_{nci}_{di}_{tt}_{h}", tag="trps")
                        nc.tensor.transpose(pst[:], hin[:, h * 128:(h + 1) * 128], ident[:])
                        nc.any.tensor_copy(xt[di * 2 + h][:, tt * 128:(tt + 1) * 128], pst[:])
            for h in range(2):
                nc.sync.dma_start(
                    xt[4 + h][:],
                    featT_d[h * 128:(h + 1) * 128, nci * NODE_CHUNK:(nci + 1) * NODE_CHUNK])

            # ---- Phase B: 3 stacked LSTM cells (h0=c0=0) ----
            x = xt  # 6 k-tiles for layer 0
            for l in range(L):
                kt = 6 if l == 0 else 2
                gates = []
                for gt in range(6):
                    ps = mm_ps.tile([128, NODE_CHUNK], f32, name=f"mmps_{nci}_{l}_{gt}", tag="mmps")
                    for k in range(kt):
                        nc.tensor.matmul(
                            out=ps[:],
                            lhsT=w_sb[l][:, k * 768 + gt * 128:k * 768 + (gt + 1) * 128],
                            rhs=x[k][:],
                            start=(k == 0), stop=(k == kt - 1),
                        )
                    gs = gate_p.tile([128, NODE_CHUNK], f32, tag="gate", name=f"gate_{nci}_{l}_{gt}")
                    nc.scalar.activation(gs[:], ps[:], Tanh if gt in (2, 3) else Sig,
                                         bias=b_sb[l][:, gt:gt + 1])
                    gates.append(gs)
                newx = []
                for h in range(2):
                    cs = cth_p.tile([128, NODE_CHUNK], f32, tag="cth", name=f"c_{nci}_{l}_{h}")
                    nc.vector.tensor_mul(cs[:], gates[h][:], gates[2 + h][:])
                    ts = cth_p.tile([128, NODE_CHUNK], f32, tag="cth", name=f"t_{nci}_{l}_{h}")
                    nc.scalar.activation(ts[:], cs[:], Tanh)
                    hs = cth_p.tile([128, NODE_CHUNK], mmdt, tag="hcast", name=f"h_{nci}_{l}_{h}", bufs=6)
                    nc.vector.tensor_mul(hs[:], gates[4 + h][:], ts[:])
                    nc.sync.dma_start(
                        csT_d[l, h * 128:(h + 1) * 128, nci * NODE_CHUNK:(nci + 1) * NODE_CHUNK], cs[:])
                    nc.sync.dma_start(
                        hsT_d[l, h * 128:(h + 1) * 128, nci * NODE_CHUNK:(nci + 1) * NODE_CHUNK], hs[:])
                    newx.append(hs)
                x = newx

        assert ci_d[0] == nchunks[0] and ci_d[1] == nchunks[1]

    nc.compile()
    return nc


def kernel(feature, edge_src, edge_dst, edge_src_rev, edge_dst_rev, h0, c0,
           W_ih0, W_hh0, b_ih0, b_hh0,
           W_ih1, W_hh1, b_ih1, b_hh1,
           W_ih2, W_hh2, b_ih2, b_hh2):
    from concourse.bass_utils import run_bass_kernel_spmd

    feature = np.ascontiguousarray(np.asarray(feature, np.float32))
    h0 = np.asarray(h0)
    c0 = np.asarray(c0)
    if np.any(h0) or np.any(c0):
        raise NotImplementedError("kernel specialized for h0=c0=0")

    idxF, dstF, wgtF, chF, tagF = _prep_direction(np.asarray(edge_src, np.int64),
                                                  np.asarray(edge_dst, np.int64))
    idxR, dstR, wgtR, chR, tagR = _prep_direction(np.asarray(edge_src_rev, np.int64),
                                                  np.asarray(edge_dst_rev, np.int64))
    cfg = {"nchunks": (len(tagF), len(tagR)), "chunks": (chF, chR),
           "tags": (tagF, tagR)}

    key = (cfg["nchunks"], tuple(chF), tuple(chR), tuple(tagF), tuple(tagR))
    if key not in _cache:
        _cache[key] = _build_program(cfg)
    nc = _cache[key]

    featT = np.zeros((NCORES, D, NPAD), np.float32)
    for c in range(NCORES):
        featT[c, :, :NPC] = feature[c * NPC:(c + 1) * NPC].T

    Ws = [np.ascontiguousarray(_keep_rows(np.asarray(W_ih0, np.float32)).T),
          np.ascontiguousarray(_keep_rows(np.asarray(W_ih1, np.float32)).T),
          np.ascontiguousarray(_keep_rows(np.asarray(W_ih2, np.float32)).T)]
    bihs = [np.ascontiguousarray(_keep_rows(np.asarray(b, np.float32).reshape(-1, 1))
                                 .reshape(6, 128).T) for b in (b_ih0, b_ih1, b_ih2)]
    bhhs = [np.ascontiguousarray(_keep_rows(np.asarray(b, np.float32).reshape(-1, 1))
                                 .reshape(6, 128).T) for b in (b_hh0, b_hh1, b_hh2)]

    fdt = np.float16 if AGG_DT == "float16" else np.float32
    feat_lo = np.ascontiguousarray(feature[:SPLIT].astype(fdt))
    feat_hi = np.ascontiguousarray(feature[SPLIT:].astype(fdt))

    in_maps = []
    for c in range(NCORES):
        m = {"feat_lo": feat_lo, "feat_hi": feat_hi, "featT": featT[c],
             "idx0": idxF[c], "dstloc0": dstF[c], "wgt0": wgtF[c],
             "idx1": idxR[c], "dstloc1": dstR[c], "wgt1": wgtR[c],
             "W0T": Ws[0], "W1T": Ws[1], "W2T": Ws[2]}
        for l in range(L):
            m[f"bih{l}"] = bihs[l]
            m[f"bhh{l}"] = bhhs[l]
        in_maps.append(m)

    res = run_bass_kernel_spmd(nc, in_maps, core_ids=list(range(NCORES)))

    hs = np.empty((L, N, D), np.float32)
    cs = np.empty((L, N, D), np.float32)
    for c in range(NCORES):
        hs[:, c * NPC:(c + 1) * NPC, :] = res.results[c]["hsT"][:, :, :NPC].transpose(0, 2, 1)
        cs[:, c * NPC:(c + 1) * NPC, :] = res.results[c]["csT"][:, :, :NPC].transpose(0, 2, 1)
    output = hs[2:3].copy()
    return output, hs, cs


# revision 9
# speedup vs baseline: 1.6644x; 1.0490x over previous
"""GCN-LSTM layer on 8 Trainium2 NeuronCores.

Strategy (graph/data parallel, no collectives):
  - Nodes sharded 8 x 6250 by dst ownership; `feature` replicated per core as
    the gather tables; LSTM weights replicated.
  - Host prep (index manipulation only): edges sorted by (dst-tile, src-half),
    bucketed per (core, 128-dst tile), padded to SPMD-identical chunk counts;
    per-edge mean weights 1/max(deg,1); own-slice features pre-transposed.
  - Device: batched dma_gather fetches source rows for runs of 128-edge
    chunks (int16 indices over two half-tables, since idx is int16); per
    chunk a scaled one-hot [128 edges, 128 dst] is built via
    iota/is_equal/mult and a TensorE matmul accumulates the mean-aggregation
    in PSUM.  Aggregates are transposed on TensorE into x^T layout
    [feat, nodes]; 3 LSTM layers (h0=c0=0 => f-gate and W_hh terms vanish)
    run as weight-stationary float32r matmuls [gates, nodes] with fused
    bias+sigmoid/tanh on ScalarE.
"""

import numpy as np

N = 50000
E = 800000
D = 256
L = 3
NCORES = 8
NPC = N // NCORES            # nodes per core = 6250
NODE_CHUNK = 512
N_NODE_CHUNKS = (NPC + NODE_CHUNK - 1) // NODE_CHUNK     # 13
NPAD = N_NODE_CHUNKS * NODE_CHUNK                        # 6656
TILES = NPAD // 128                                      # 52 dst tiles / core
SPLIT = 32768                # feature table split so local idx fits int16
M_G = 8                      # max chunks per dma_gather (1024 idxs = SWDGE ring cap)
G_NBUF = 4
MM_DT = "float16"            # matmul compute dtype (LSTM path)
AGG_DT = "float16"           # gather-table / aggregation matmul dtype

_cache = {}


def _prep_direction(src, dst):
    """Sort edges by (owning 128-dst tile, src-half), pad each tile's lo/hi
    chunk counts to the max over cores (SPMD program identity).

    Returns per-core arrays:
      idx16  [NCORES, 128, NCHUNK*8] int16  (dma_gather layout: flat edge
              slot i lives at [16*g + i%16, i//16] for all g in 0..7)
      dstloc [NCORES, 128, NCHUNK] f32  (dst within tile, -1 for padding)
      wgt    [NCORES, 128, NCHUNK] f32  (1/max(deg,1), 0 for padding)
    plus shared chunk metadata: chunks_per_tile list and per-chunk table
    tags (0=lo table, 1=hi table).
    """
    deg = np.bincount(dst, minlength=N)
    w_node = (1.0 / np.maximum(deg, 1)).astype(np.float32)

    core = dst // NPC
    tloc = (dst - core * NPC) // 128
    tkey = core * TILES + tloc
    hi = (src >= SPLIT).astype(np.int8)
    order = np.lexsort((hi, tkey))
    src_s = src[order]
    dst_s = dst[order]
    tkey_s = tkey[order]
    hi_s = hi[order]
    w_s = w_node[dst_s]

    bounds = np.searchsorted(tkey_s, np.arange(NCORES * TILES + 1))
    hicum = np.concatenate([[0], np.cumsum(hi_s)])
    n_tot = (bounds[1:] - bounds[:-1]).reshape(NCORES, TILES)
    n_hi = (hicum[bounds[1:]] - hicum[bounds[:-1]]).reshape(NCORES, TILES)
    n_lo = n_tot - n_hi

    ch_lo = (-(-n_lo // 128)).max(axis=0)
    ch_hi = (-(-n_hi // 128)).max(axis=0)
    empty = (ch_lo + ch_hi) == 0
    ch_lo[empty] = 1
    nchunk = int((ch_lo + ch_hi).sum())

    tags = []
    for t in range(TILES):
        tags += [0] * int(ch_lo[t]) + [1] * int(ch_hi[t])
    chunks_per_tile = [int(ch_lo[t] + ch_hi[t]) for t in range(TILES)]

    slot_starts = np.concatenate([[0], np.cumsum((ch_lo + ch_hi) * 128)])
    srcloc = np.zeros((NCORES, nchunk * 128), np.int32)
    dstloc = np.full((NCORES, nchunk * 128), -1.0, np.float32)
    wgt = np.zeros((NCORES, nchunk * 128), np.float32)
    for c in range(NCORES):
        base = c * NPC
        for t in range(TILES):
            e0 = bounds[c * TILES + t]
            nl = int(n_lo[c, t])
            nh = int(n_hi[c, t])
            s_lo = int(slot_starts[t])
            s_hi = s_lo + int(ch_lo[t]) * 128
            for (es, n, ss, off) in ((e0, nl, s_lo, 0), (e0 + nl, nh, s_hi, SPLIT)):
                if n == 0:
                    continue
                srcloc[c, ss:ss + n] = src_s[es:es + n] - off
                dstloc[c, ss:ss + n] = (dst_s[es:es + n] - base - t * 128).astype(np.float32)
                wgt[c, ss:ss + n] = w_s[es:es + n]

    # [128, NCHUNK] with [p, ci] = edge slot ci*128+p
    def interleave(a):
        return np.ascontiguousarray(a.reshape(NCORES, nchunk, 128).transpose(0, 2, 1))

    dstloc = interleave(dstloc)
    wgt = interleave(wgt)
    assert srcloc.max() < 32768
    # dma_gather int16 layout: flat slot i at [i%16, i//16], tiled to 128 rows
    base16 = srcloc.reshape(NCORES, nchunk * 8, 16).transpose(0, 2, 1).astype(np.int16)
    idx16 = np.ascontiguousarray(np.tile(base16, (1, 8, 1)))
    return idx16, dstloc, wgt, chunks_per_tile, tags


def _make_groups(tags):
    """Runs of same-table chunks, capped at M_G: list of (start, n, tag)."""
    groups = []
    i = 0
    while i < len(tags):
        j = i
        while j < len(tags) and tags[j] == tags[i] and j - i < M_G:
            j += 1
        groups.append((i, j - i, tags[i]))
        i = j
    return groups


def _keep_rows(w):
    # PyTorch gate order i,f,g,o; f unused when c0=0 -> keep i,g,o
    return np.concatenate([w[0:256], w[512:1024]], axis=0)


def _build_program(cfg):
    import contextlib
    import concourse.tile as tile
    from concourse import bacc, mybir
    from concourse.masks import make_identity

    f32 = mybir.dt.float32
    i16 = mybir.dt.int16
    i32 = mybir.dt.int32
    mmdt = getattr(mybir.dt, MM_DT)
    aggdt = getattr(mybir.dt, AGG_DT)

    nchunks, chunks, tags = cfg["nchunks"], cfg["chunks"], cfg["tags"]
    groups = [_make_groups(tags[di]) for di in range(2)]
    # chunk -> (group index, slot in group, is_first_chunk_of_group)
    ch2grp = []
    for di in range(2):
        m = {}
        for gi, (st, n, _tag) in enumerate(groups[di]):
            for k in range(n):
                m[st + k] = (gi, k, k == 0)
        ch2grp.append(m)

    nc = bacc.Bacc("TRN2", target_bir_lowering=False)

    flo_d = nc.dram_tensor("feat_lo", [SPLIT, D], aggdt, kind="ExternalInput")
    fhi_d = nc.dram_tensor("feat_hi", [N - SPLIT, D], aggdt, kind="ExternalInput")
    featT_d = nc.dram_tensor("featT", [D, NPAD], mmdt, kind="ExternalInput")
    idx_d, dst_d, wgt_d = [], [], []
    for di in range(2):
        idx_d.append(nc.dram_tensor(f"idx{di}", [128, nchunks[di] * 8], i16, kind="ExternalInput"))
        dst_d.append(nc.dram_tensor(f"dstloc{di}", [128, nchunks[di]], f32, kind="ExternalInput"))
        wgt_d.append(nc.dram_tensor(f"wgt{di}", [128, nchunks[di]], f32, kind="ExternalInput"))
    wT_d = [nc.dram_tensor("W0T", [768, 768], mmdt, kind="ExternalInput"),
            nc.dram_tensor("W1T", [D, 768], mmdt, kind="ExternalInput"),
            nc.dram_tensor("W2T", [D, 768], mmdt, kind="ExternalInput")]
    bih_d = [nc.dram_tensor(f"bih{l}", [128, 6], f32, kind="ExternalInput") for l in range(L)]
    bhh_d = [nc.dram_tensor(f"bhh{l}", [128, 6], f32, kind="ExternalInput") for l in range(L)]
    hsT_d = nc.dram_tensor("hsT", [L, D, NPAD], mmdt, kind="ExternalOutput")
    csT_d = nc.dram_tensor("csT", [L, D, NPAD], mmdt, kind="ExternalOutput")

    Sig = mybir.ActivationFunctionType.Sigmoid
    Tanh = mybir.ActivationFunctionType.Tanh

    with tile.TileContext(nc) as tc, contextlib.ExitStack() as ctx:
        const_p = ctx.enter_context(tc.tile_pool(name="const", bufs=1))
        g_p = ctx.enter_context(tc.tile_pool(name="g", bufs=1))
        oh_p = ctx.enter_context(tc.tile_pool(name="oh", bufs=6))
        hin_p = ctx.enter_context(tc.tile_pool(name="hin", bufs=3))
        xt_p = ctx.enter_context(tc.tile_pool(name="xt", bufs=2))
        gate_p = ctx.enter_context(tc.tile_pool(name="gate", bufs=7))
        cth_p = ctx.enter_context(tc.tile_pool(name="cth", bufs=10))
        agg_ps = ctx.enter_context(tc.tile_pool(name="aggps", bufs=3, space="PSUM"))
        tr_ps = ctx.enter_context(tc.tile_pool(name="trps", bufs=2, space="PSUM"))
        mm_ps = ctx.enter_context(tc.tile_pool(name="mmps", bufs=3, space="PSUM"))

        ident = const_p.tile([128, 128], mmdt, tag="ident")
        make_identity(nc, ident[:])
        iota_i = const_p.tile([128, 128], i32, tag="iota_i")
        nc.gpsimd.iota(iota_i[:], pattern=[[1, 128]], base=0, channel_multiplier=0)
        iota_f = const_p.tile([128, 128], aggdt, tag="iota_f")
        nc.vector.tensor_copy(iota_f[:], iota_i[:])

        # LSTM weights -> SBUF, k-tile major: w[l][:, k*768 + gt*128 :]
        w_sb = []
        for l in range(L):
            kt = 6 if l == 0 else 2
            w = const_p.tile([128, kt * 768], mmdt, name=f"w{l}", tag=f"w{l}")
            for k in range(kt):
                nc.sync.dma_start(w[:, k * 768:(k + 1) * 768], wT_d[l][k * 128:(k + 1) * 128, :])
            w_sb.append(w)
        b_sb = []
        for l in range(L):
            bi = const_p.tile([128, 6], f32, name=f"bi{l}", tag=f"bi{l}")
            nc.sync.dma_start(bi[:], bih_d[l][:])
            bh = const_p.tile([128, 6], f32, name=f"bh{l}", tag=f"bh{l}")
            nc.sync.dma_start(bh[:], bhh_d[l][:])
            b = const_p.tile([128, 6], f32, name=f"b{l}", tag=f"b{l}")
            nc.vector.tensor_add(b[:], bi[:], bh[:])
            b_sb.append(b)

        # metadata tiles loaded in per-node-chunk slices (overlap with compute)
        idx_sb, dst_sb, wgt_sb = [], [], []
        for di in range(2):
            s = const_p.tile([128, nchunks[di] * 8], i16, name=f"idx{di}", tag=f"idx{di}")
            idx_sb.append(s)
            dl = const_p.tile([128, nchunks[di]], f32, name=f"dst{di}", tag=f"dst{di}")
            dst_sb.append(dl)
            w = const_p.tile([128, nchunks[di]], f32, name=f"wgt{di}", tag=f"wgtsb{di}")
            wgt_sb.append(w)
        # per-(nci, di) chunk ranges
        meta_edges = [[0], [0]]
        for _nci in range(N_NODE_CHUNKS):
            for _di in range(2):
                meta_edges[_di].append(meta_edges[_di][-1]
                                       + sum(chunks[_di][_nci * 4:(_nci + 1) * 4]))

        def load_meta(nci):
            for di in range(2):
                a, b = meta_edges[di][nci], meta_edges[di][nci + 1]
                if b == a:
                    continue
                nc.sync.dma_start(idx_sb[di][:, a * 8:b * 8], idx_d[di][:, a * 8:b * 8])
                nc.sync.dma_start(dst_sb[di][:, a:b], dst_d[di][:, a:b])
                nc.sync.dma_start(wgt_sb[di][:, a:b], wgt_d[di][:, a:b])

        G_bufs = [g_p.tile([128, M_G, D], aggdt, tag=f"G{i}", name=f"G{i}") for i in range(G_NBUF)]

        def emit_gather(di, gi):
            st, n, tag = groups[di][gi]
            table = flo_d if tag == 0 else fhi_d
            nc.gpsimd.dma_gather(
                out_ap=G_bufs[gi % G_NBUF][:, :n, :],
                in_ap=table[:],
                idxs_ap=idx_sb[di][:, st * 8:(st + n) * 8],
                num_idxs=n * 128,
                num_idxs_reg=n * 128,
                elem_size=D,
            )

        ci_d = [0, 0]
        for nci in range(N_NODE_CHUNKS):
            load_meta(nci)
            # ---- Phase A: aggregation for this chunk's 4 dst tiles ----
            xt = [xt_p.tile([128, NODE_CHUNK], mmdt, tag=f"xt{k}", name=f"xt{k}_{nci}")
                  for k in range(6)]
            for di in range(2):
                for tt in range(4):
                    tg = nci * 4 + tt
                    psum = agg_ps.tile([128, D], f32, name=f"aggps_{nci}_{di}_{tt}", tag="aggps")
                    nch = chunks[di][tg]
                    for k in range(nch):
                        ci = ci_d[di]
                        gi, slot, first = ch2grp[di][ci]
                        if first:
                            emit_gather(di, gi)
                        oh = oh_p.tile([128, 128], aggdt, tag="oh", name=f"oh_{di}_{ci}")
                        nc.any.tensor_scalar(
                            out=oh[:], in0=iota_f[:],
                            scalar1=dst_sb[di][:, ci:ci + 1],
                            scalar2=wgt_sb[di][:, ci:ci + 1],
                            op0=mybir.AluOpType.is_equal, op1=mybir.AluOpType.mult,
                        )
                        nc.tensor.matmul(
                            out=psum[:],
                            lhsT=oh[:],
                            rhs=G_bufs[gi % G_NBUF][:, slot, :],
                            start=(k == 0), stop=(k == nch - 1),
                        )
                        ci_d[di] += 1
                    hin = hin_p.tile([128, D], mmdt, tag="hin", name=f"hin_{nci}_{di}_{tt}")
                    nc.any.tensor_copy(hin[:], psum[:])
                    for h in range(2):
                        pst = tr_ps.tile([128, 128], mmdt, name=f"trps_{nci}_{di}_{tt}_{h}", tag="trps")
                        nc.tensor.transpose(pst[:], hin[:, h * 128:(h + 1) * 128], ident[:])
                        nc.any.tensor_copy(xt[di * 2 + h][:, tt * 128:(tt + 1) * 128], pst[:])
            for h in range(2):
                nc.sync.dma_start(
                    xt[4 + h][:],
                    featT_d[h * 128:(h + 1) * 128, nci * NODE_CHUNK:(nci + 1) * NODE_CHUNK])

            # ---- Phase B: 3 stacked LSTM cells (h0=c0=0) ----
            x = xt  # 6 k-tiles for layer 0
            for l in range(L):
                kt = 6 if l == 0 else 2
                gates = []
                for gt in range(6):
                    ps = mm_ps.tile([128, NODE_CHUNK], f32, name=f"mmps_{nci}_{l}_{gt}", tag="mmps")
                    for k in range(kt):
                        nc.tensor.matmul(
                            out=ps[:],
                            lhsT=w_sb[l][:, k * 768 + gt * 128:k * 768 + (gt + 1) * 128],
                            rhs=x[k][:],
                            start=(k == 0), stop=(k == kt - 1),
                        )
                    gs = gate_p.tile([128, NODE_CHUNK], f32, tag="gate", name=f"gate_{nci}_{l}_{gt}")
                    nc.scalar.activation(gs[:], ps[:], Tanh if gt in (2, 3) else Sig,
                                         bias=b_sb[l][:, gt:gt + 1])
                    gates.append(gs)
                newx = []
                for h in range(2):
                    cs = cth_p.tile([128, NODE_CHUNK], mmdt, tag="cth", name=f"c_{nci}_{l}_{h}")
                    nc.vector.tensor_mul(cs[:], gates[h][:], gates[2 + h][:])
                    ts = cth_p.tile([128, NODE_CHUNK], f32, tag="cth", name=f"t_{nci}_{l}_{h}")
                    nc.scalar.activation(ts[:], cs[:], Tanh)
                    hs = cth_p.tile([128, NODE_CHUNK], mmdt, tag="hcast", name=f"h_{nci}_{l}_{h}", bufs=6)
                    nc.vector.tensor_mul(hs[:], gates[4 + h][:], ts[:])
                    nc.sync.dma_start(
                        csT_d[l, h * 128:(h + 1) * 128, nci * NODE_CHUNK:(nci + 1) * NODE_CHUNK], cs[:])
                    nc.sync.dma_start(
                        hsT_d[l, h * 128:(h + 1) * 128, nci * NODE_CHUNK:(nci + 1) * NODE_CHUNK], hs[:])
                    newx.append(hs)
                x = newx

        assert ci_d[0] == nchunks[0] and ci_d[1] == nchunks[1]

    nc.compile()
    return nc


def kernel(feature, edge_src, edge_dst, edge_src_rev, edge_dst_rev, h0, c0,
           W_ih0, W_hh0, b_ih0, b_hh0,
           W_ih1, W_hh1, b_ih1, b_hh1,
           W_ih2, W_hh2, b_ih2, b_hh2):
    from concourse.bass_utils import run_bass_kernel_spmd

    feature = np.ascontiguousarray(np.asarray(feature, np.float32))
    h0 = np.asarray(h0)
    c0 = np.asarray(c0)
    if np.any(h0) or np.any(c0):
        raise NotImplementedError("kernel specialized for h0=c0=0")

    idxF, dstF, wgtF, chF, tagF = _prep_direction(np.asarray(edge_src, np.int64),
                                                  np.asarray(edge_dst, np.int64))
    idxR, dstR, wgtR, chR, tagR = _prep_direction(np.asarray(edge_src_rev, np.int64),
                                                  np.asarray(edge_dst_rev, np.int64))
    cfg = {"nchunks": (len(tagF), len(tagR)), "chunks": (chF, chR),
           "tags": (tagF, tagR)}

    key = (cfg["nchunks"], tuple(chF), tuple(chR), tuple(tagF), tuple(tagR))
    if key not in _cache:
        _cache[key] = _build_program(cfg)
    nc = _cache[key]

    wdt = np.float16 if MM_DT == "float16" else np.float32
    featT = np.zeros((NCORES, D, NPAD), wdt)
    for c in range(NCORES):
        featT[c, :, :NPC] = feature[c * NPC:(c + 1) * NPC].T.astype(wdt)

    Ws = [np.ascontiguousarray(_keep_rows(np.asarray(W_ih0, np.float32)).T.astype(wdt)),
          np.ascontiguousarray(_keep_rows(np.asarray(W_ih1, np.float32)).T.astype(wdt)),
          np.ascontiguousarray(_keep_rows(np.asarray(W_ih2, np.float32)).T.astype(wdt))]
    bihs = [np.ascontiguousarray(_keep_rows(np.asarray(b, np.float32).reshape(-1, 1))
                                 .reshape(6, 128).T) for b in (b_ih0, b_ih1, b_ih2)]
    bhhs = [np.ascontiguousarray(_keep_rows(np.asarray(b, np.float32).reshape(-1, 1))
                                 .reshape(6, 128).T) for b in (b_hh0, b_hh1, b_hh2)]

    fdt = np.float16 if AGG_DT == "float16" else np.float32
    feat_lo = np.ascontiguousarray(feature[:SPLIT].astype(fdt))
    feat_hi = np.ascontiguousarray(feature[SPLIT:].astype(fdt))

    in_maps = []
    for c in range(NCORES):
        m = {"feat_lo": feat_lo, "feat_hi": feat_hi, "featT": featT[c],
             "idx0": idxF[c], "dstloc0": dstF[c], "wgt0": wgtF[c],
             "idx1": idxR[c], "dstloc1": dstR[c], "wgt1": wgtR[c],
             "W0T": Ws[0], "W1T": Ws[1], "W2T": Ws[2]}
        for l in range(L):
            m[f"bih{l}"] = bihs[l]
            m[f"bhh{l}"] = bhhs[l]
        in_maps.append(m)

    res = run_bass_kernel_spmd(nc, in_maps, core_ids=list(range(NCORES)))

    hs = np.empty((L, N, D), np.float32)
    cs = np.empty((L, N, D), np.float32)
    for c in range(NCORES):
        hs[:, c * NPC:(c + 1) * NPC, :] = res.results[c]["hsT"][:, :, :NPC].transpose(0, 2, 1).astype(np.float32)
        cs[:, c * NPC:(c + 1) * NPC, :] = res.results[c]["csT"][:, :, :NPC].transpose(0, 2, 1).astype(np.float32)
    output = hs[2:3].copy()
    return output, hs, cs


# revision 10
# speedup vs baseline: 1.7825x; 1.0709x over previous
"""GCN-LSTM layer on 8 Trainium2 NeuronCores.

Strategy (graph/data parallel, no collectives):
  - Nodes sharded 8 x 6250 by dst ownership; `feature` replicated per core as
    the gather tables; LSTM weights replicated.
  - Host prep (index manipulation only): edges sorted by (dst-tile, src-half),
    bucketed per (core, 128-dst tile), padded to SPMD-identical chunk counts;
    per-edge mean weights 1/max(deg,1); own-slice features pre-transposed.
  - Device: batched dma_gather fetches source rows for runs of 128-edge
    chunks (int16 indices over two half-tables, since idx is int16); per
    chunk a scaled one-hot [128 edges, 128 dst] is built via
    iota/is_equal/mult and a TensorE matmul accumulates the mean-aggregation
    in PSUM.  Aggregates are transposed on TensorE into x^T layout
    [feat, nodes]; 3 LSTM layers (h0=c0=0 => f-gate and W_hh terms vanish)
    run as weight-stationary float32r matmuls [gates, nodes] with fused
    bias+sigmoid/tanh on ScalarE.
"""

import numpy as np

N = 50000
E = 800000
D = 256
L = 3
NCORES = 8
NPC = N // NCORES            # nodes per core = 6250
NODE_CHUNK = 512
N_NODE_CHUNKS = (NPC + NODE_CHUNK - 1) // NODE_CHUNK     # 13
NPAD = N_NODE_CHUNKS * NODE_CHUNK                        # 6656
TILES = NPAD // 128                                      # 52 dst tiles / core
SPLIT = 32768                # feature table split so local idx fits int16
M_G = 8                      # max chunks per dma_gather (1024 idxs = SWDGE ring cap)
G_NBUF = 6
MM_DT = "float16"            # matmul compute dtype (LSTM path)
AGG_DT = "float16"           # gather-table / aggregation matmul dtype

_cache = {}


def _prep_direction(src, dst):
    """Sort edges by (owning 128-dst tile, src-half), pad each tile's lo/hi
    chunk counts to the max over cores (SPMD program identity).

    Returns per-core arrays:
      idx16  [NCORES, 128, NCHUNK*8] int16  (dma_gather layout: flat edge
              slot i lives at [16*g + i%16, i//16] for all g in 0..7)
      dstloc [NCORES, 128, NCHUNK] f32  (dst within tile, -1 for padding)
      wgt    [NCORES, 128, NCHUNK] f32  (1/max(deg,1), 0 for padding)
    plus shared chunk metadata: chunks_per_tile list and per-chunk table
    tags (0=lo table, 1=hi table).
    """
    deg = np.bincount(dst, minlength=N)
    w_node = (1.0 / np.maximum(deg, 1)).astype(np.float32)

    core = dst // NPC
    tloc = (dst - core * NPC) // 128
    tkey = core * TILES + tloc
    hi = (src >= SPLIT).astype(np.int8)
    order = np.lexsort((hi, tkey))
    src_s = src[order]
    dst_s = dst[order]
    tkey_s = tkey[order]
    hi_s = hi[order]
    w_s = w_node[dst_s]

    bounds = np.searchsorted(tkey_s, np.arange(NCORES * TILES + 1))
    hicum = np.concatenate([[0], np.cumsum(hi_s)])
    n_tot = (bounds[1:] - bounds[:-1]).reshape(NCORES, TILES)
    n_hi = (hicum[bounds[1:]] - hicum[bounds[:-1]]).reshape(NCORES, TILES)
    n_lo = n_tot - n_hi

    ch_lo = (-(-n_lo // 128)).max(axis=0)
    ch_hi = (-(-n_hi // 128)).max(axis=0)
    empty = (ch_lo + ch_hi) == 0
    ch_lo[empty] = 1
    nchunk = int((ch_lo + ch_hi).sum())

    tags = []
    for t in range(TILES):
        tags += [0] * int(ch_lo[t]) + [1] * int(ch_hi[t])
    chunks_per_tile = [int(ch_lo[t] + ch_hi[t]) for t in range(TILES)]

    slot_starts = np.concatenate([[0], np.cumsum((ch_lo + ch_hi) * 128)])
    srcloc = np.zeros((NCORES, nchunk * 128), np.int32)
    dstloc = np.full((NCORES, nchunk * 128), -1.0, np.float32)
    wgt = np.zeros((NCORES, nchunk * 128), np.float32)
    for c in range(NCORES):
        base = c * NPC
        for t in range(TILES):
            e0 = bounds[c * TILES + t]
            nl = int(n_lo[c, t])
            nh = int(n_hi[c, t])
            s_lo = int(slot_starts[t])
            s_hi = s_lo + int(ch_lo[t]) * 128
            for (es, n, ss, off) in ((e0, nl, s_lo, 0), (e0 + nl, nh, s_hi, SPLIT)):
                if n == 0:
                    continue
                srcloc[c, ss:ss + n] = src_s[es:es + n] - off
                dstloc[c, ss:ss + n] = (dst_s[es:es + n] - base - t * 128).astype(np.float32)
                wgt[c, ss:ss + n] = w_s[es:es + n]

    # [128, NCHUNK] with [p, ci] = edge slot ci*128+p
    def interleave(a):
        return np.ascontiguousarray(a.reshape(NCORES, nchunk, 128).transpose(0, 2, 1))

    dstloc = interleave(dstloc)
    wgt = interleave(wgt)
    assert srcloc.max() < 32768
    # dma_gather int16 layout: flat slot i at [i%16, i//16], tiled to 128 rows
    base16 = srcloc.reshape(NCORES, nchunk * 8, 16).transpose(0, 2, 1).astype(np.int16)
    idx16 = np.ascontiguousarray(np.tile(base16, (1, 8, 1)))
    return idx16, dstloc, wgt, chunks_per_tile, tags


def _make_groups(tags):
    """Runs of same-table chunks, capped at M_G: list of (start, n, tag)."""
    groups = []
    i = 0
    while i < len(tags):
        j = i
        while j < len(tags) and tags[j] == tags[i] and j - i < M_G:
            j += 1
        groups.append((i, j - i, tags[i]))
        i = j
    return groups


def _keep_rows(w):
    # PyTorch gate order i,f,g,o; f unused when c0=0 -> keep i,g,o
    return np.concatenate([w[0:256], w[512:1024]], axis=0)


def _build_program(cfg):
    import contextlib
    import concourse.tile as tile
    from concourse import bacc, mybir
    from concourse.masks import make_identity

    f32 = mybir.dt.float32
    i16 = mybir.dt.int16
    i32 = mybir.dt.int32
    mmdt = getattr(mybir.dt, MM_DT)
    aggdt = getattr(mybir.dt, AGG_DT)

    nchunks, chunks, tags = cfg["nchunks"], cfg["chunks"], cfg["tags"]
    groups = [_make_groups(tags[di]) for di in range(2)]
    # chunk -> (group index, slot in group, is_first_chunk_of_group)
    ch2grp = []
    for di in range(2):
        m = {}
        for gi, (st, n, _tag) in enumerate(groups[di]):
            for k in range(n):
                m[st + k] = (gi, k, k == 0)
        ch2grp.append(m)

    nc = bacc.Bacc("TRN2", target_bir_lowering=False)

    flo_d = nc.dram_tensor("feat_lo", [SPLIT, D], aggdt, kind="ExternalInput")
    fhi_d = nc.dram_tensor("feat_hi", [N - SPLIT, D], aggdt, kind="ExternalInput")
    featT_d = nc.dram_tensor("featT", [D, NPAD], mmdt, kind="ExternalInput")
    idx_d, dst_d, wgt_d = [], [], []
    for di in range(2):
        idx_d.append(nc.dram_tensor(f"idx{di}", [128, nchunks[di] * 8], i16, kind="ExternalInput"))
        dst_d.append(nc.dram_tensor(f"dstloc{di}", [128, nchunks[di]], f32, kind="ExternalInput"))
        wgt_d.append(nc.dram_tensor(f"wgt{di}", [128, nchunks[di]], f32, kind="ExternalInput"))
    wT_d = [nc.dram_tensor("W0T", [768, 768], mmdt, kind="ExternalInput"),
            nc.dram_tensor("W1T", [D, 768], mmdt, kind="ExternalInput"),
            nc.dram_tensor("W2T", [D, 768], mmdt, kind="ExternalInput")]
    bih_d = [nc.dram_tensor(f"bih{l}", [128, 6], f32, kind="ExternalInput") for l in range(L)]
    bhh_d = [nc.dram_tensor(f"bhh{l}", [128, 6], f32, kind="ExternalInput") for l in range(L)]
    hsT_d = nc.dram_tensor("hsT", [L, D, NPAD], mmdt, kind="ExternalOutput")
    csT_d = nc.dram_tensor("csT", [L, D, NPAD], mmdt, kind="ExternalOutput")

    Sig = mybir.ActivationFunctionType.Sigmoid
    Tanh = mybir.ActivationFunctionType.Tanh

    with tile.TileContext(nc) as tc, contextlib.ExitStack() as ctx:
        const_p = ctx.enter_context(tc.tile_pool(name="const", bufs=1))
        g_p = ctx.enter_context(tc.tile_pool(name="g", bufs=1))
        oh_p = ctx.enter_context(tc.tile_pool(name="oh", bufs=8))
        hin_p = ctx.enter_context(tc.tile_pool(name="hin", bufs=4))
        xt_p = ctx.enter_context(tc.tile_pool(name="xt", bufs=3))
        gate_p = ctx.enter_context(tc.tile_pool(name="gate", bufs=8))
        cth_p = ctx.enter_context(tc.tile_pool(name="cth", bufs=12))
        agg_ps = ctx.enter_context(tc.tile_pool(name="aggps", bufs=3, space="PSUM"))
        tr_ps = ctx.enter_context(tc.tile_pool(name="trps", bufs=2, space="PSUM"))
        mm_ps = ctx.enter_context(tc.tile_pool(name="mmps", bufs=3, space="PSUM"))

        ident = const_p.tile([128, 128], mmdt, tag="ident")
        make_identity(nc, ident[:])
        iota_i = const_p.tile([128, 128], i32, tag="iota_i")
        nc.gpsimd.iota(iota_i[:], pattern=[[1, 128]], base=0, channel_multiplier=0)
        iota_f = const_p.tile([128, 128], aggdt, tag="iota_f")
        nc.vector.tensor_copy(iota_f[:], iota_i[:])

        # LSTM weights -> SBUF, k-tile major: w[l][:, k*768 + gt*128 :]
        w_sb = []
        for l in range(L):
            kt = 6 if l == 0 else 2
            w = const_p.tile([128, kt * 768], mmdt, name=f"w{l}", tag=f"w{l}")
            for k in range(kt):
                nc.sync.dma_start(w[:, k * 768:(k + 1) * 768], wT_d[l][k * 128:(k + 1) * 128, :])
            w_sb.append(w)
        b_sb = []
        for l in range(L):
            bi = const_p.tile([128, 6], f32, name=f"bi{l}", tag=f"bi{l}")
            nc.sync.dma_start(bi[:], bih_d[l][:])
            bh = const_p.tile([128, 6], f32, name=f"bh{l}", tag=f"bh{l}")
            nc.sync.dma_start(bh[:], bhh_d[l][:])
            b = const_p.tile([128, 6], f32, name=f"b{l}", tag=f"b{l}")
            nc.vector.tensor_add(b[:], bi[:], bh[:])
            b_sb.append(b)

        # metadata tiles loaded in per-node-chunk slices (overlap with compute)
        idx_sb, dst_sb, wgt_sb = [], [], []
        for di in range(2):
            s = const_p.tile([128, nchunks[di] * 8], i16, name=f"idx{di}", tag=f"idx{di}")
            idx_sb.append(s)
            dl = const_p.tile([128, nchunks[di]], f32, name=f"dst{di}", tag=f"dst{di}")
            dst_sb.append(dl)
            w = const_p.tile([128, nchunks[di]], f32, name=f"wgt{di}", tag=f"wgtsb{di}")
            wgt_sb.append(w)
        # per-(nci, di) chunk ranges
        meta_edges = [[0], [0]]
        for _nci in range(N_NODE_CHUNKS):
            for _di in range(2):
                meta_edges[_di].append(meta_edges[_di][-1]
                                       + sum(chunks[_di][_nci * 4:(_nci + 1) * 4]))

        def load_meta(nci):
            for di in range(2):
                a, b = meta_edges[di][nci], meta_edges[di][nci + 1]
                if b == a:
                    continue
                nc.sync.dma_start(idx_sb[di][:, a * 8:b * 8], idx_d[di][:, a * 8:b * 8])
                nc.sync.dma_start(dst_sb[di][:, a:b], dst_d[di][:, a:b])
                nc.sync.dma_start(wgt_sb[di][:, a:b], wgt_d[di][:, a:b])

        G_bufs = [g_p.tile([128, M_G, D], aggdt, tag=f"G{i}", name=f"G{i}") for i in range(G_NBUF)]

        def emit_gather(di, gi):
            st, n, tag = groups[di][gi]
            table = flo_d if tag == 0 else fhi_d
            nc.gpsimd.dma_gather(
                out_ap=G_bufs[gi % G_NBUF][:, :n, :],
                in_ap=table[:],
                idxs_ap=idx_sb[di][:, st * 8:(st + n) * 8],
                num_idxs=n * 128,
                num_idxs_reg=n * 128,
                elem_size=D,
            )

        ci_d = [0, 0]
        for nci in range(N_NODE_CHUNKS):
            load_meta(nci)
            # ---- Phase A: aggregation for this chunk's 4 dst tiles ----
            xt = [xt_p.tile([128, NODE_CHUNK], mmdt, tag=f"xt{k}", name=f"xt{k}_{nci}")
                  for k in range(6)]
            for di in range(2):
                for tt in range(4):
                    tg = nci * 4 + tt
                    psum = agg_ps.tile([128, D], f32, name=f"aggps_{nci}_{di}_{tt}", tag="aggps")
                    nch = chunks[di][tg]
                    for k in range(nch):
                        ci = ci_d[di]
                        gi, slot, first = ch2grp[di][ci]
                        if first:
                            emit_gather(di, gi)
                        oh = oh_p.tile([128, 128], aggdt, tag="oh", name=f"oh_{di}_{ci}")
                        nc.any.tensor_scalar(
                            out=oh[:], in0=iota_f[:],
                            scalar1=dst_sb[di][:, ci:ci + 1],
                            scalar2=wgt_sb[di][:, ci:ci + 1],
                            op0=mybir.AluOpType.is_equal, op1=mybir.AluOpType.mult,
                        )
                        nc.tensor.matmul(
                            out=psum[:],
                            lhsT=oh[:],
                            rhs=G_bufs[gi % G_NBUF][:, slot, :],
                            start=(k == 0), stop=(k == nch - 1),
                        )
                        ci_d[di] += 1
                    hin = hin_p.tile([128, D], mmdt, tag="hin", name=f"hin_{nci}_{di}_{tt}")
                    nc.any.tensor_copy(hin[:], psum[:])
                    for h in range(2):
                        pst = tr_ps.tile([128, 128], mmdt, name=f"trps_{nci}_{di}_{tt}_{h}", tag="trps")
                        nc.tensor.transpose(pst[:], hin[:, h * 128:(h + 1) * 128], ident[:])
                        nc.any.tensor_copy(xt[di * 2 + h][:, tt * 128:(tt + 1) * 128], pst[:])
            for h in range(2):
                nc.sync.dma_start(
                    xt[4 + h][:],
                    featT_d[h * 128:(h + 1) * 128, nci * NODE_CHUNK:(nci + 1) * NODE_CHUNK])

            # ---- Phase B: 3 stacked LSTM cells (h0=c0=0) ----
            x = xt  # 6 k-tiles for layer 0
            for l in range(L):
                kt = 6 if l == 0 else 2
                gates = []
                for gt in range(6):
                    ps = mm_ps.tile([128, NODE_CHUNK], f32, name=f"mmps_{nci}_{l}_{gt}", tag="mmps")
                    for k in range(kt):
                        nc.tensor.matmul(
                            out=ps[:],
                            lhsT=w_sb[l][:, k * 768 + gt * 128:k * 768 + (gt + 1) * 128],
                            rhs=x[k][:],
                            start=(k == 0), stop=(k == kt - 1),
                        )
                    gs = gate_p.tile([128, NODE_CHUNK], f32, tag="gate", name=f"gate_{nci}_{l}_{gt}")
                    nc.scalar.activation(gs[:], ps[:], Tanh if gt in (2, 3) else Sig,
                                         bias=b_sb[l][:, gt:gt + 1])
                    gates.append(gs)
                newx = []
                for h in range(2):
                    cs = cth_p.tile([128, NODE_CHUNK], mmdt, tag="cth", name=f"c_{nci}_{l}_{h}")
                    nc.vector.tensor_mul(cs[:], gates[h][:], gates[2 + h][:])
                    ts = cth_p.tile([128, NODE_CHUNK], f32, tag="cth", name=f"t_{nci}_{l}_{h}")
                    nc.scalar.activation(ts[:], cs[:], Tanh)
                    hs = cth_p.tile([128, NODE_CHUNK], mmdt, tag="hcast", name=f"h_{nci}_{l}_{h}", bufs=6)
                    nc.vector.tensor_mul(hs[:], gates[4 + h][:], ts[:])
                    nc.sync.dma_start(
                        csT_d[l, h * 128:(h + 1) * 128, nci * NODE_CHUNK:(nci + 1) * NODE_CHUNK], cs[:])
                    nc.sync.dma_start(
                        hsT_d[l, h * 128:(h + 1) * 128, nci * NODE_CHUNK:(nci + 1) * NODE_CHUNK], hs[:])
                    newx.append(hs)
                x = newx

        assert ci_d[0] == nchunks[0] and ci_d[1] == nchunks[1]

    nc.compile()
    return nc


def kernel(feature, edge_src, edge_dst, edge_src_rev, edge_dst_rev, h0, c0,
           W_ih0, W_hh0, b_ih0, b_hh0,
           W_ih1, W_hh1, b_ih1, b_hh1,
           W_ih2, W_hh2, b_ih2, b_hh2):
    from concourse.bass_utils import run_bass_kernel_spmd

    feature = np.ascontiguousarray(np.asarray(feature, np.float32))
    h0 = np.asarray(h0)
    c0 = np.asarray(c0)
    if np.any(h0) or np.any(c0):
        raise NotImplementedError("kernel specialized for h0=c0=0")

    idxF, dstF, wgtF, chF, tagF = _prep_direction(np.asarray(edge_src, np.int64),
                                                  np.asarray(edge_dst, np.int64))
    idxR, dstR, wgtR, chR, tagR = _prep_direction(np.asarray(edge_src_rev, np.int64),
                                                  np.asarray(edge_dst_rev, np.int64))
    cfg = {"nchunks": (len(tagF), len(tagR)), "chunks": (chF, chR),
           "tags": (tagF, tagR)}

    key = (cfg["nchunks"], tuple(chF), tuple(chR), tuple(tagF), tuple(tagR))
    if key not in _cache:
        _cache[key] = _build_program(cfg)
    nc = _cache[key]

    wdt = np.float16 if MM_DT == "float16" else np.float32
    featT = np.zeros((NCORES, D, NPAD), wdt)
    for c in range(NCORES):
        featT[c, :, :NPC] = feature[c * NPC:(c + 1) * NPC].T.astype(wdt)

    Ws = [np.ascontiguousarray(_keep_rows(np.asarray(W_ih0, np.float32)).T.astype(wdt)),
          np.ascontiguousarray(_keep_rows(np.asarray(W_ih1, np.float32)).T.astype(wdt)),
          np.ascontiguousarray(_keep_rows(np.asarray(W_ih2, np.float32)).T.astype(wdt))]
    bihs = [np.ascontiguousarray(_keep_rows(np.asarray(b, np.float32).reshape(-1, 1))
                                 .reshape(6, 128).T) for b in (b_ih0, b_ih1, b_ih2)]
    bhhs = [np.ascontiguousarray(_keep_rows(np.asarray(b, np.float32).reshape(-1, 1))
                                 .reshape(6, 128).T) for b in (b_hh0, b_hh1, b_hh2)]

    fdt = np.float16 if AGG_DT == "float16" else np.float32
    feat_lo = np.ascontiguousarray(feature[:SPLIT].astype(fdt))
    feat_hi = np.ascontiguousarray(feature[SPLIT:].astype(fdt))

    in_maps = []
    for c in range(NCORES):
        m = {"feat_lo": feat_lo, "feat_hi": feat_hi, "featT": featT[c],
             "idx0": idxF[c], "dstloc0": dstF[c], "wgt0": wgtF[c],
             "idx1": idxR[c], "dstloc1": dstR[c], "wgt1": wgtR[c],
             "W0T": Ws[0], "W1T": Ws[1], "W2T": Ws[2]}
        for l in range(L):
            m[f"bih{l}"] = bihs[l]
            m[f"bhh{l}"] = bhhs[l]
        in_maps.append(m)

    res = run_bass_kernel_spmd(nc, in_maps, core_ids=list(range(NCORES)))

    hs = np.empty((L, N, D), np.float32)
    cs = np.empty((L, N, D), np.float32)
    for c in range(NCORES):
        hs[:, c * NPC:(c + 1) * NPC, :] = res.results[c]["hsT"][:, :, :NPC].transpose(0, 2, 1).astype(np.float32)
        cs[:, c * NPC:(c + 1) * NPC, :] = res.results[c]["csT"][:, :, :NPC].transpose(0, 2, 1).astype(np.float32)
    output = hs[2:3].copy()
    return output, hs, cs


# revision 11
# speedup vs baseline: 1.8685x; 1.0483x over previous
"""GCN-LSTM layer on 8 Trainium2 NeuronCores.

Strategy (graph/data parallel, no collectives):
  - Nodes sharded 8 x 6250 by dst ownership; `feature` replicated per core as
    the gather tables; LSTM weights replicated.
  - Host prep (index manipulation only): edges sorted by (dst-tile, src-half),
    bucketed per (core, 128-dst tile), padded to SPMD-identical chunk counts;
    per-edge mean weights 1/max(deg,1); own-slice features pre-transposed.
  - Device: batched dma_gather fetches source rows for runs of 128-edge
    chunks (int16 indices over two half-tables, since idx is int16); per
    chunk a scaled one-hot [128 edges, 128 dst] is built via
    iota/is_equal/mult and a TensorE matmul accumulates the mean-aggregation
    in PSUM.  Aggregates are transposed on TensorE into x^T layout
    [feat, nodes]; 3 LSTM layers (h0=c0=0 => f-gate and W_hh terms vanish)
    run as weight-stationary float32r matmuls [gates, nodes] with fused
    bias+sigmoid/tanh on ScalarE.
"""

import numpy as np

N = 50000
E = 800000
D = 256
L = 3
NCORES = 8
NPC = N // NCORES            # nodes per core = 6250
NODE_CHUNK = 512
N_NODE_CHUNKS = (NPC + NODE_CHUNK - 1) // NODE_CHUNK     # 13
NPAD = N_NODE_CHUNKS * NODE_CHUNK                        # 6656
TILES = NPAD // 128                                      # 52 dst tiles / core
SPLIT = 32768                # feature table split so local idx fits int16
M_G = 8                      # max chunks per dma_gather (1024 idxs = SWDGE ring cap)
G_NBUF = 8
MM_DT = "float16"            # matmul compute dtype (LSTM path)
AGG_DT = "float16"           # gather-table / aggregation matmul dtype

_cache = {}


def _prep_direction(src, dst):
    """Sort edges by (owning 128-dst tile, src-half), pad each tile's lo/hi
    chunk counts to the max over cores (SPMD program identity).

    Returns per-core arrays:
      idx16  [NCORES, 128, NCHUNK*8] int16  (dma_gather layout: flat edge
              slot i lives at [16*g + i%16, i//16] for all g in 0..7)
      dstloc [NCORES, 128, NCHUNK] f32  (dst within tile, -1 for padding)
      wgt    [NCORES, 128, NCHUNK] f32  (1/max(deg,1), 0 for padding)
    plus shared chunk metadata: chunks_per_tile list and per-chunk table
    tags (0=lo table, 1=hi table).
    """
    deg = np.bincount(dst, minlength=N)
    w_node = (1.0 / np.maximum(deg, 1)).astype(np.float32)

    core = dst // NPC
    tloc = (dst - core * NPC) // 128
    tkey = core * TILES + tloc
    hi = (src >= SPLIT).astype(np.int8)
    order = np.lexsort((hi, tkey))
    src_s = src[order]
    dst_s = dst[order]
    tkey_s = tkey[order]
    hi_s = hi[order]
    w_s = w_node[dst_s]

    bounds = np.searchsorted(tkey_s, np.arange(NCORES * TILES + 1))
    hicum = np.concatenate([[0], np.cumsum(hi_s)])
    n_tot = (bounds[1:] - bounds[:-1]).reshape(NCORES, TILES)
    n_hi = (hicum[bounds[1:]] - hicum[bounds[:-1]]).reshape(NCORES, TILES)
    n_lo = n_tot - n_hi

    ch_lo = (-(-n_lo // 128)).max(axis=0)
    ch_hi = (-(-n_hi // 128)).max(axis=0)
    empty = (ch_lo + ch_hi) == 0
    ch_lo[empty] = 1
    nchunk = int((ch_lo + ch_hi).sum())

    tags = []
    for t in range(TILES):
        tags += [0] * int(ch_lo[t]) + [1] * int(ch_hi[t])
    chunks_per_tile = [int(ch_lo[t] + ch_hi[t]) for t in range(TILES)]

    slot_starts = np.concatenate([[0], np.cumsum((ch_lo + ch_hi) * 128)])
    srcloc = np.zeros((NCORES, nchunk * 128), np.int32)
    dstloc = np.full((NCORES, nchunk * 128), -1.0, np.float32)
    wgt = np.zeros((NCORES, nchunk * 128), np.float32)
    for c in range(NCORES):
        base = c * NPC
        for t in range(TILES):
            e0 = bounds[c * TILES + t]
            nl = int(n_lo[c, t])
            nh = int(n_hi[c, t])
            s_lo = int(slot_starts[t])
            s_hi = s_lo + int(ch_lo[t]) * 128
            for (es, n, ss, off) in ((e0, nl, s_lo, 0), (e0 + nl, nh, s_hi, SPLIT)):
                if n == 0:
                    continue
                srcloc[c, ss:ss + n] = src_s[es:es + n] - off
                dstloc[c, ss:ss + n] = (dst_s[es:es + n] - base - t * 128).astype(np.float32)
                wgt[c, ss:ss + n] = w_s[es:es + n]

    # [128, NCHUNK] with [p, ci] = edge slot ci*128+p
    def interleave(a):
        return np.ascontiguousarray(a.reshape(NCORES, nchunk, 128).transpose(0, 2, 1))

    dstloc = interleave(dstloc)
    wgt = interleave(wgt)
    assert srcloc.max() < 32768
    # dma_gather int16 layout: flat slot i at [i%16, i//16], tiled to 128 rows
    base16 = srcloc.reshape(NCORES, nchunk * 8, 16).transpose(0, 2, 1).astype(np.int16)
    idx16 = np.ascontiguousarray(np.tile(base16, (1, 8, 1)))
    return idx16, dstloc, wgt, chunks_per_tile, tags


def _make_groups(tags):
    """Runs of same-table chunks, capped at M_G: list of (start, n, tag)."""
    groups = []
    i = 0
    while i < len(tags):
        j = i
        while j < len(tags) and tags[j] == tags[i] and j - i < M_G:
            j += 1
        groups.append((i, j - i, tags[i]))
        i = j
    return groups


def _keep_rows(w):
    # PyTorch gate order i,f,g,o; f unused when c0=0 -> keep i,g,o
    return np.concatenate([w[0:256], w[512:1024]], axis=0)


def _build_program(cfg):
    import contextlib
    import concourse.tile as tile
    from concourse import bacc, mybir
    from concourse.masks import make_identity

    f32 = mybir.dt.float32
    i16 = mybir.dt.int16
    i32 = mybir.dt.int32
    mmdt = getattr(mybir.dt, MM_DT)
    aggdt = getattr(mybir.dt, AGG_DT)

    nchunks, chunks, tags = cfg["nchunks"], cfg["chunks"], cfg["tags"]
    groups = [_make_groups(tags[di]) for di in range(2)]
    # chunk -> (group index, slot in group, is_first_chunk_of_group)
    ch2grp = []
    for di in range(2):
        m = {}
        for gi, (st, n, _tag) in enumerate(groups[di]):
            for k in range(n):
                m[st + k] = (gi, k, k == 0)
        ch2grp.append(m)

    nc = bacc.Bacc("TRN2", target_bir_lowering=False)

    flo_d = nc.dram_tensor("feat_lo", [SPLIT, D], aggdt, kind="ExternalInput")
    fhi_d = nc.dram_tensor("feat_hi", [N - SPLIT, D], aggdt, kind="ExternalInput")
    featT_d = nc.dram_tensor("featT", [D, NPAD], mmdt, kind="ExternalInput")
    idx_d, dst_d, wgt_d = [], [], []
    for di in range(2):
        idx_d.append(nc.dram_tensor(f"idx{di}", [128, nchunks[di] * 8], i16, kind="ExternalInput"))
        dst_d.append(nc.dram_tensor(f"dstloc{di}", [128, nchunks[di]], f32, kind="ExternalInput"))
        wgt_d.append(nc.dram_tensor(f"wgt{di}", [128, nchunks[di]], f32, kind="ExternalInput"))
    wT_d = [nc.dram_tensor("W0T", [768, 768], mmdt, kind="ExternalInput"),
            nc.dram_tensor("W1T", [D, 768], mmdt, kind="ExternalInput"),
            nc.dram_tensor("W2T", [D, 768], mmdt, kind="ExternalInput")]
    bih_d = [nc.dram_tensor(f"bih{l}", [128, 6], f32, kind="ExternalInput") for l in range(L)]
    bhh_d = [nc.dram_tensor(f"bhh{l}", [128, 6], f32, kind="ExternalInput") for l in range(L)]
    hsT_d = nc.dram_tensor("hsT", [L, D, NPAD], mmdt, kind="ExternalOutput")
    csT_d = nc.dram_tensor("csT", [L, D, NPAD], mmdt, kind="ExternalOutput")

    Sig = mybir.ActivationFunctionType.Sigmoid
    Tanh = mybir.ActivationFunctionType.Tanh

    with tile.TileContext(nc) as tc, contextlib.ExitStack() as ctx:
        const_p = ctx.enter_context(tc.tile_pool(name="const", bufs=1))
        g_p = ctx.enter_context(tc.tile_pool(name="g", bufs=1))
        oh_p = ctx.enter_context(tc.tile_pool(name="oh", bufs=10))
        hin_p = ctx.enter_context(tc.tile_pool(name="hin", bufs=6))
        xt_p = ctx.enter_context(tc.tile_pool(name="xt", bufs=4))
        gate_p = ctx.enter_context(tc.tile_pool(name="gate", bufs=10))
        cth_p = ctx.enter_context(tc.tile_pool(name="cth", bufs=14))
        agg_ps = ctx.enter_context(tc.tile_pool(name="aggps", bufs=3, space="PSUM"))
        tr_ps = ctx.enter_context(tc.tile_pool(name="trps", bufs=2, space="PSUM"))
        mm_ps = ctx.enter_context(tc.tile_pool(name="mmps", bufs=3, space="PSUM"))

        ident = const_p.tile([128, 128], mmdt, tag="ident")
        make_identity(nc, ident[:])
        iota_i = const_p.tile([128, 128], i32, tag="iota_i")
        nc.gpsimd.iota(iota_i[:], pattern=[[1, 128]], base=0, channel_multiplier=0)
        iota_f = const_p.tile([128, 128], aggdt, tag="iota_f")
        nc.vector.tensor_copy(iota_f[:], iota_i[:])

        # LSTM weights -> SBUF, k-tile major: w[l][:, k*768 + gt*128 :]
        w_sb = []
        for l in range(L):
            kt = 6 if l == 0 else 2
            w = const_p.tile([128, kt * 768], mmdt, name=f"w{l}", tag=f"w{l}")
            for k in range(kt):
                nc.sync.dma_start(w[:, k * 768:(k + 1) * 768], wT_d[l][k * 128:(k + 1) * 128, :])
            w_sb.append(w)
        b_sb = []
        for l in range(L):
            bi = const_p.tile([128, 6], f32, name=f"bi{l}", tag=f"bi{l}")
            nc.sync.dma_start(bi[:], bih_d[l][:])
            bh = const_p.tile([128, 6], f32, name=f"bh{l}", tag=f"bh{l}")
            nc.sync.dma_start(bh[:], bhh_d[l][:])
            b = const_p.tile([128, 6], f32, name=f"b{l}", tag=f"b{l}")
            nc.vector.tensor_add(b[:], bi[:], bh[:])
            b_sb.append(b)

        # metadata tiles loaded in per-node-chunk slices (overlap with compute)
        idx_sb, dst_sb, wgt_sb = [], [], []
        for di in range(2):
            s = const_p.tile([128, nchunks[di] * 8], i16, name=f"idx{di}", tag=f"idx{di}")
            idx_sb.append(s)
            dl = const_p.tile([128, nchunks[di]], f32, name=f"dst{di}", tag=f"dst{di}")
            dst_sb.append(dl)
            w = const_p.tile([128, nchunks[di]], f32, name=f"wgt{di}", tag=f"wgtsb{di}")
            wgt_sb.append(w)
        # per-(nci, di) chunk ranges
        meta_edges = [[0], [0]]
        for _nci in range(N_NODE_CHUNKS):
            for _di in range(2):
                meta_edges[_di].append(meta_edges[_di][-1]
                                       + sum(chunks[_di][_nci * 4:(_nci + 1) * 4]))

        def load_meta(nci):
            for di in range(2):
                a, b = meta_edges[di][nci], meta_edges[di][nci + 1]
                if b == a:
                    continue
                nc.sync.dma_start(idx_sb[di][:, a * 8:b * 8], idx_d[di][:, a * 8:b * 8])
                nc.sync.dma_start(dst_sb[di][:, a:b], dst_d[di][:, a:b])
                nc.sync.dma_start(wgt_sb[di][:, a:b], wgt_d[di][:, a:b])

        G_bufs = [g_p.tile([128, M_G, D], aggdt, tag=f"G{i}", name=f"G{i}") for i in range(G_NBUF)]

        def emit_gather(di, gi):
            st, n, tag = groups[di][gi]
            table = flo_d if tag == 0 else fhi_d
            nc.gpsimd.dma_gather(
                out_ap=G_bufs[gi % G_NBUF][:, :n, :],
                in_ap=table[:],
                idxs_ap=idx_sb[di][:, st * 8:(st + n) * 8],
                num_idxs=n * 128,
                num_idxs_reg=n * 128,
                elem_size=D,
            )

        ci_d = [0, 0]
        for nci in range(N_NODE_CHUNKS):
            load_meta(nci)
            # ---- Phase A: aggregation for this chunk's 4 dst tiles ----
            xt = [xt_p.tile([128, NODE_CHUNK], mmdt, tag=f"xt{k}", name=f"xt{k}_{nci}")
                  for k in range(6)]
            for di in range(2):
                for tt in range(4):
                    tg = nci * 4 + tt
                    psum = agg_ps.tile([128, D], f32, name=f"aggps_{nci}_{di}_{tt}", tag="aggps")
                    nch = chunks[di][tg]
                    for k in range(nch):
                        ci = ci_d[di]
                        gi, slot, first = ch2grp[di][ci]
                        if first:
                            emit_gather(di, gi)
                        oh = oh_p.tile([128, 128], aggdt, tag="oh", name=f"oh_{di}_{ci}")
                        nc.any.tensor_scalar(
                            out=oh[:], in0=iota_f[:],
                            scalar1=dst_sb[di][:, ci:ci + 1],
                            scalar2=wgt_sb[di][:, ci:ci + 1],
                            op0=mybir.AluOpType.is_equal, op1=mybir.AluOpType.mult,
                        )
                        nc.tensor.matmul(
                            out=psum[:],
                            lhsT=oh[:],
                            rhs=G_bufs[gi % G_NBUF][:, slot, :],
                            start=(k == 0), stop=(k == nch - 1),
                        )
                        ci_d[di] += 1
                    hin = hin_p.tile([128, D], mmdt, tag="hin", name=f"hin_{nci}_{di}_{tt}")
                    nc.any.tensor_copy(hin[:], psum[:])
                    for h in range(2):
                        pst = tr_ps.tile([128, 128], mmdt, name=f"trps_{nci}_{di}_{tt}_{h}", tag="trps")
                        nc.tensor.transpose(pst[:], hin[:, h * 128:(h + 1) * 128], ident[:])
                        nc.any.tensor_copy(xt[di * 2 + h][:, tt * 128:(tt + 1) * 128], pst[:])
            for h in range(2):
                nc.sync.dma_start(
                    xt[4 + h][:],
                    featT_d[h * 128:(h + 1) * 128, nci * NODE_CHUNK:(nci + 1) * NODE_CHUNK])

            # ---- Phase B: 3 stacked LSTM cells (h0=c0=0) ----
            x = xt  # 6 k-tiles for layer 0
            for l in range(L):
                kt = 6 if l == 0 else 2
                gates = []
                for gt in range(6):
                    ps = mm_ps.tile([128, NODE_CHUNK], f32, name=f"mmps_{nci}_{l}_{gt}", tag="mmps")
                    for k in range(kt):
                        nc.tensor.matmul(
                            out=ps[:],
                            lhsT=w_sb[l][:, k * 768 + gt * 128:k * 768 + (gt + 1) * 128],
                            rhs=x[k][:],
                            start=(k == 0), stop=(k == kt - 1),
                        )
                    gs = gate_p.tile([128, NODE_CHUNK], f32, tag="gate", name=f"gate_{nci}_{l}_{gt}")
                    nc.scalar.activation(gs[:], ps[:], Tanh if gt in (2, 3) else Sig,
                                         bias=b_sb[l][:, gt:gt + 1])
                    gates.append(gs)
                newx = []
                for h in range(2):
                    cs = cth_p.tile([128, NODE_CHUNK], mmdt, tag="cth", name=f"c_{nci}_{l}_{h}")
                    nc.vector.tensor_mul(cs[:], gates[h][:], gates[2 + h][:])
                    ts = cth_p.tile([128, NODE_CHUNK], f32, tag="cth", name=f"t_{nci}_{l}_{h}")
                    nc.scalar.activation(ts[:], cs[:], Tanh)
                    hs = cth_p.tile([128, NODE_CHUNK], mmdt, tag="hcast", name=f"h_{nci}_{l}_{h}", bufs=6)
                    nc.vector.tensor_mul(hs[:], gates[4 + h][:], ts[:])
                    nc.sync.dma_start(
                        csT_d[l, h * 128:(h + 1) * 128, nci * NODE_CHUNK:(nci + 1) * NODE_CHUNK], cs[:])
                    nc.sync.dma_start(
                        hsT_d[l, h * 128:(h + 1) * 128, nci * NODE_CHUNK:(nci + 1) * NODE_CHUNK], hs[:])
                    newx.append(hs)
                x = newx

        assert ci_d[0] == nchunks[0] and ci_d[1] == nchunks[1]

    nc.compile()
    return nc


def kernel(feature, edge_src, edge_dst, edge_src_rev, edge_dst_rev, h0, c0,
           W_ih0, W_hh0, b_ih0, b_hh0,
           W_ih1, W_hh1, b_ih1, b_hh1,
           W_ih2, W_hh2, b_ih2, b_hh2):
    from concourse.bass_utils import run_bass_kernel_spmd

    feature = np.ascontiguousarray(np.asarray(feature, np.float32))
    h0 = np.asarray(h0)
    c0 = np.asarray(c0)
    if np.any(h0) or np.any(c0):
        raise NotImplementedError("kernel specialized for h0=c0=0")

    idxF, dstF, wgtF, chF, tagF = _prep_direction(np.asarray(edge_src, np.int64),
                                                  np.asarray(edge_dst, np.int64))
    idxR, dstR, wgtR, chR, tagR = _prep_direction(np.asarray(edge_src_rev, np.int64),
                                                  np.asarray(edge_dst_rev, np.int64))
    cfg = {"nchunks": (len(tagF), len(tagR)), "chunks": (chF, chR),
           "tags": (tagF, tagR)}

    key = (cfg["nchunks"], tuple(chF), tuple(chR), tuple(tagF), tuple(tagR))
    if key not in _cache:
        _cache[key] = _build_program(cfg)
    nc = _cache[key]

    wdt = np.float16 if MM_DT == "float16" else np.float32
    featT = np.zeros((NCORES, D, NPAD), wdt)
    for c in range(NCORES):
        featT[c, :, :NPC] = feature[c * NPC:(c + 1) * NPC].T.astype(wdt)

    Ws = [np.ascontiguousarray(_keep_rows(np.asarray(W_ih0, np.float32)).T.astype(wdt)),
          np.ascontiguousarray(_keep_rows(np.asarray(W_ih1, np.float32)).T.astype(wdt)),
          np.ascontiguousarray(_keep_rows(np.asarray(W_ih2, np.float32)).T.astype(wdt))]
    bihs = [np.ascontiguousarray(_keep_rows(np.asarray(b, np.float32).reshape(-1, 1))
                                 .reshape(6, 128).T) for b in (b_ih0, b_ih1, b_ih2)]
    bhhs = [np.ascontiguousarray(_keep_rows(np.asarray(b, np.float32).reshape(-1, 1))
                                 .reshape(6, 128).T) for b in (b_hh0, b_hh1, b_hh2)]

    fdt = np.float16 if AGG_DT == "float16" else np.float32
    feat_lo = np.ascontiguousarray(feature[:SPLIT].astype(fdt))
    feat_hi = np.ascontiguousarray(feature[SPLIT:].astype(fdt))

    in_maps = []
    for c in range(NCORES):
        m = {"feat_lo": feat_lo, "feat_hi": feat_hi, "featT": featT[c],
             "idx0": idxF[c], "dstloc0": dstF[c], "wgt0": wgtF[c],
             "idx1": idxR[c], "dstloc1": dstR[c], "wgt1": wgtR[c],
             "W0T": Ws[0], "W1T": Ws[1], "W2T": Ws[2]}
        for l in range(L):
            m[f"bih{l}"] = bihs[l]
            m[f"bhh{l}"] = bhhs[l]
        in_maps.append(m)

    res = run_bass_kernel_spmd(nc, in_maps, core_ids=list(range(NCORES)))

    hs = np.empty((L, N, D), np.float32)
    cs = np.empty((L, N, D), np.float32)
    for c in range(NCORES):
        hs[:, c * NPC:(c + 1) * NPC, :] = res.results[c]["hsT"][:, :, :NPC].transpose(0, 2, 1).astype(np.float32)
        cs[:, c * NPC:(c + 1) * NPC, :] = res.results[c]["csT"][:, :, :NPC].transpose(0, 2, 1).astype(np.float32)
    output = hs[2:3].copy()
    return output, hs, cs
